# revision 16
# baseline (speedup 1.0000x reference)
"""Multi-head self-attention on 8 Trainium2 NeuronCores (Bass/Tile).

Problem: x[2,2048,1024] -> MHA(16 heads, d_head 64) -> out[2,2048,1024].

Sharding (batch x head-group, Megatron-ish, collective-free):
  core c (0..7): batch b = c//4, head group g = c%4 (heads 4g..4g+3).
  Each core computes q/k/v projections for its 4 heads over its batch,
  attention for those heads, and a PARTIAL output projection
  attn_local[256ch] @ w_out[256ch rows] over the full sequence. The host
  sums the 4 partials per batch (the Megatron row-parallel all-reduce is
  folded into the unshard step; b_out and the V-bias term bv @ w_out are
  added once on the host -- exact, since softmax rows sum to 1).

On-core layout (TensorE compute in bf16, fp32 PSUM accumulation):
  - ACT (exp for softmax) is the bottleneck engine: 16.8M exps/core ~=
    147us of ACT instruction time. The schedule saturates ACT from the
    earliest possible point after the ~7us engine-boot preamble:
    * all inputs arrive in host-pre-blocked layouts that are contiguous
      per SBUF partition (cheap HWDGE triggers, 4-8KB DMA lines), split
      across the sync and scalar queues with x^T token-sliced;
    * k/q chunk-0 projections are emitted first so scores round 0 feeds
      ACT immediately; V projections and the remaining q/k chunks are
      pure PE filler behind ACT pacing.
  - exp p-tiles are paired per round ([128, 2heads, 8kp, 1024]) with 3
    rotating buffers = 3 rounds in flight, so round r+2's exps never
    wait on round r's PV consumption.
  - qT/kT in [channel, t] layout: scores^T = kT.T @ qT with the two
    heads of a chunk in partitions 0-63/64-127 -> concurrent K=64
    matmuls in disjoint PE row groups.
  - softmax: scores^T [128ki, qi] -> ACT exp (PSUM->SBUF bf16,
    scale=1/8 folded, no max subtraction: |s|/8 <= ~2).
  - PV: attn^T = V.T @ P~ as column-tiled concurrent M=64 head pairs;
    denominators via DVE bf16 add-tree + K=128 ones-matmul fold,
    reciprocal_approx_fast straight off PSUM; normalize one round
    behind PV (rep-matmul broadcast + DVE mul); out-projection +
    output DMA per query group.
"""

import numpy as np
import ml_dtypes

import concourse.bass as bass
import concourse.mybir as mybir
import concourse.tile as tile
from concourse import bacc
from concourse import bass_utils
from concourse.bass import ts

BF = mybir.dt.bfloat16
F32 = mybir.dt.float32

B, T, C = 2, 2048, 1024
H, DH = 16, 64
N_CORES = 8
HG = 4  # heads per core
CH = HG * DH  # 256 channels per core

LAST_RESULT = None  # BassKernelResults of the most recent run (for profiling)
_NC_CACHE = None


def _build_nc():
    nc = bacc.Bacc(
        "TRN2", target_bir_lowering=False, debug=False, num_devices=N_CORES
    )

    # host-pre-blocked layouts: every tensor is contiguous along its SBUF
    # partition's free dim, so each DMA is 128 x (one fat line).
    xt = nc.dram_tensor("xt", [128, 4, 8, 512], BF, kind="ExternalInput")
    wq = nc.dram_tensor("wq", [128, 8, CH], BF, kind="ExternalInput")
    wk = nc.dram_tensor("wk", [128, 8, CH], BF, kind="ExternalInput")
    wv = nc.dram_tensor("wv", [128, 8, CH], BF, kind="ExternalInput")
    bqt = nc.dram_tensor("bqt", [128, 2], F32, kind="ExternalInput")
    bkt = nc.dram_tensor("bkt", [128, 2], F32, kind="ExternalInput")
    wout = nc.dram_tensor("wout", [128, 2, C], BF, kind="ExternalInput")
    out = nc.dram_tensor("out", [T, C], F32, kind="ExternalOutput")

    with tile.TileContext(nc) as tc:
        with (
            tc.tile_pool(name="persist", bufs=1) as persist,
            tc.tile_pool(name="consts", bufs=1) as consts,
            tc.tile_pool(name="sbn", bufs=2) as sbn,
            tc.tile_pool(name="osb", bufs=3) as osb,
            tc.tile_pool(name="ps_st", bufs=2, space="PSUM") as ps_st,
            tc.tile_pool(name="ps_pv", bufs=2, space="PSUM") as ps_pv,
            tc.tile_pool(name="ps_misc", bufs=2, space="PSUM") as ps_misc,
        ):
            ones_bf = consts.tile([1, 128], BF)
            nc.vector.memset(ones_bf[:], 1.0)
            ones_col = consts.tile([128, 1], BF)
            nc.vector.memset(ones_col[:], 1.0)

            # xT is token-major: [p, tt(512-token block), ci, t-within-block]
            xT = persist.tile([128, 4, 8, 512], BF, tag="xT")
            wq_sb = persist.tile([128, 8, CH], BF, tag="wq")
            wk_sb = persist.tile([128, 8, CH], BF, tag="wk")
            wv_sb = persist.tile([128, 8, CH], BF, tag="wv")
            wout_sb = persist.tile([128, 2, C], BF, tag="wout")
            bqt_sb = consts.tile([128, 2], F32)
            bkt_sb = consts.tile([128, 2], F32)

            # ---- input DMA: sync + scalar HWDGE queues, critical-first ----
            nc.sync.dma_start(out=wk_sb[:], in_=wk[:])
            nc.scalar.dma_start(out=bkt_sb[:], in_=bkt[:])
            nc.scalar.dma_start(out=bqt_sb[:], in_=bqt[:])
            nc.scalar.dma_start(out=wq_sb[:], in_=wq[:])
            nc.sync.dma_start(out=xT[:, 0], in_=xt[:, 0])
            nc.scalar.dma_start(out=xT[:, 1], in_=xt[:, 1])
            nc.sync.dma_start(out=xT[:, 2], in_=xt[:, 2])
            nc.scalar.dma_start(out=xT[:, 3], in_=xt[:, 3])
            nc.gpsimd.dma_start(out=wv_sb[:], in_=wv[:])
            nc.gpsimd.dma_start(out=wout_sb[:], in_=wout[:])

            # ---- persistent activations ----
            # qkT[:, 0:2, :] = qT chunks (hp), [:, 2:4, :] = kT chunks;
            # chunk hp rows 0-63 = head 2hp, rows 64-127 = head 2hp+1.
            qkT = persist.tile([128, 4, T], BF, tag="qkT")
            vext = persist.tile([128, T // 128, HG, DH], BF, tag="vext")
            attn_p = [
                [
                    persist.tile(
                        [128, 512], BF, tag=f"attnp{hp}_{qg}",
                        name=f"attnp{hp}_{qg}",
                    )
                    for qg in range(4)
                ]
                for hp in range(2)
            ]

            def qk_group(w_i, co, tt):
                """one [128,512] tile of qT (w_i=0) or kT (w_i=1), chunk co"""
                wsb = wq_sb if w_i == 0 else wk_sb
                bias_sb = bqt_sb if w_i == 0 else bkt_sb
                qp = ps_misc.tile([128, 512], F32, tag="sm", name="qp")
                for ci in range(8):
                    nc.tensor.matmul(
                        qp[:],
                        wsb[:, ci, ts(co, 128)],
                        xT[:, tt, ci, :],
                        start=(ci == 0),
                        stop=(ci == 7),
                    )
                # bias-add + cast on the DVE (keeps the ACT queue for exps)
                nc.vector.tensor_scalar_add(
                    qkT[:, 2 * w_i + co, ts(tt, 512)],
                    qp[:],
                    bias_sb[:, co : co + 1],
                )

            def v_group(tv):
                vp = ps_misc.tile([128, CH], F32, tag="sm", name="vp")
                for ci in range(8):
                    nc.tensor.matmul(
                        vp[:],
                        xT[:, tv // 4, ci, ts(tv % 4, 128)],
                        wv_sb[:, ci, :],
                        start=(ci == 0),
                        stop=(ci == 7),
                    )
                nc.vector.tensor_copy(
                    vext[:, tv, :, :],
                    vp[:].rearrange("p (h d) -> p h d", h=HG),
                )

            p_tiles = {}
            rec_tiles = {}
            tmp_tiles = {}

            def p_alloc(qg, hp):
                # paired tile: [p, head(A/B), kp, 1024]; one pool slot per
                # round -> bufs=3 keeps 3 rounds of exps live.
                p = osb.tile([128, 2, 8, 1024], BF, tag="p", bufs=3, name="p")
                p_tiles[(qg, hp)] = p
                return p

            def st_seg(qg, hp, kps, p):
                """scores^T + exp for head pair hp, query group qg, kp range."""
                qs = ts(qg, 512)
                for kp in kps:
                    stA = ps_st.tile([128, 1024], F32, tag="st", name="stA")
                    stB = ps_st.tile([128, 1024], F32, tag="st", name="stB")
                    for j in range(2):
                        ki = 2 * kp + j
                        nc.tensor.matmul(
                            stA[:, ts(j, 512)],
                            qkT[0:64, 2 + hp, ts(ki, 128)],
                            qkT[0:64, hp, qs],
                            start=True, stop=True,
                        )
                        nc.tensor.matmul(
                            stB[:, ts(j, 512)],
                            qkT[64:128, 2 + hp, ts(ki, 128)],
                            qkT[64:128, hp, qs],
                            start=True, stop=True,
                        )
                    nc.scalar.activation(
                        p[:, 0, kp, :], stA[:],
                        mybir.ActivationFunctionType.Exp, scale=1.0 / 8.0,
                    )
                    nc.scalar.activation(
                        p[:, 1, kp, :], stB[:],
                        mybir.ActivationFunctionType.Exp, scale=1.0 / 8.0,
                    )

            def st_part(qg, hp):
                p = p_alloc(qg, hp)
                st_seg(qg, hp, range(8), p)

            def pv_part(qg, hp):
                p = p_tiles.pop((qg, hp))
                # denominator add-trees first: they depend only on the exps,
                # so emitting them ahead of the PV matmuls keeps the DVE
                # queue from head-blocking on the PV-dependent tmp copy.
                # t2 runs on the (otherwise idle) GPSIMD except in the final
                # rounds, where the tail's serial chain wants the faster DVE.
                t2_eng = nc.vector if (qg, hp) >= (3, 0) else nc.gpsimd
                t4s = {}
                for hh in range(2):
                    t1 = sbn.tile([128, 4, 1024], BF, tag="t1", name="t1", bufs=1)
                    nc.vector.tensor_add(
                        t1[:], p[:, hh, 0:4, :], p[:, hh, 4:8, :]
                    )
                    t2 = sbn.tile([128, 2, 1024], BF, tag="t2", name="t2", bufs=1)
                    t2_eng.tensor_add(
                        t2[:], t1[:, 0:2, :], t1[:, 2:4, :]
                    )
                    t3 = sbn.tile([128, 1024], BF, tag="t3", name="t3", bufs=1)
                    nc.vector.tensor_add(
                        t3[:], t2[:, 0, :], t2[:, 1, :]
                    )
                    t4 = sbn.tile([128, 512], BF, tag="t4", name="t4", bufs=2)
                    nc.vector.tensor_add(
                        t4[:], t3[:, 0:512], t3[:, 512:1024]
                    )
                    t4s[hh] = t4
                # paired PV: head 2hp -> psum partitions 0-63 (col group 0-1),
                # head 2hp+1 -> partitions 64-127 (col group 2-3); the two
                # column-tiled matmul streams run concurrently on the PE.
                pv = ps_pv.tile([128, 512], F32, tag="pv", name="pv")
                for ki in range(16):
                    for hh in range(2):
                        h = 2 * hp + hh
                        nc.tensor.matmul(
                            pv[64 * hh : 64 * hh + 64, :],
                            vext[:, ki, h, :],
                            p[:, hh, ki // 2, ts(ki % 2, 512)],
                            start=(ki == 0),
                            stop=(ki == 15),
                        )
                # partition-axis fold of the partial denominators (K=128
                # ones-matmul), then the reciprocal chain (DVE-only)
                for hh in range(2):
                    h = 2 * hp + hh
                    dps = ps_misc.tile([128, 512], F32, tag="sm", name="dps")
                    nc.tensor.matmul(
                        dps[0:1, :], ones_col[:, 0:1], t4s[hh][:],
                        start=True, stop=True,
                    )
                    rec32 = sbn.tile([1, 512], F32, tag="rec32", name="rc", bufs=1)
                    nc.vector.reciprocal_approx_fast(out=rec32[:], in_=dps[0:1, :])
                    rec_bf = sbn.tile([1, 512], BF, tag="rec", name="rb", bufs=4)
                    nc.vector.tensor_copy(rec_bf[:], rec32[:])
                    rec_tiles[4 * qg + h] = rec_bf
                tmp = sbn.tile([128, 512], BF, tag="tmp", name="tmp", bufs=4)
                nc.scalar.copy(tmp[:], pv[:])
                tmp_tiles[(qg, hp)] = tmp

            def normalize_round(qg, hp):
                """rep-matmul + multiply -> attn_p[hp][qg] (both heads)."""
                rp = ps_misc.tile([128, 512], F32, tag="sm", name="rp")
                tmp = tmp_tiles.pop((qg, hp))
                for hh in range(2):
                    slot = 4 * qg + 2 * hp + hh
                    rows = slice(64 * hh, 64 * hh + 64)
                    nc.tensor.matmul(
                        rp[rows, :], ones_bf[0:1, 0:64], rec_tiles[slot][:],
                        start=True, stop=True,
                    )
                    nc.vector.tensor_mul(
                        attn_p[hp][qg][rows, :],
                        tmp[rows, :],
                        rp[rows, :],
                    )

            def outproj_chunk(qg):
                """partial out-projection rows for query group qg."""
                for tt4 in range(4):
                    tt = 4 * qg + tt4
                    o_sb = osb.tile([128, C], F32, tag="o", name="osb", bufs=1)
                    for cn in range(2):
                        op = ps_misc.tile(
                            [128, 512], F32, tag="sm", name="op"
                        )
                        for hp in range(2):
                            nc.tensor.matmul(
                                op[:],
                                attn_p[hp][qg][:, ts(tt4, 128)],
                                wout_sb[:, hp, ts(cn, 512)],
                                start=(hp == 0),
                                stop=(hp == 1),
                            )
                        nc.scalar.copy(o_sb[:, ts(cn, 512)], op[:])
                    nc.sync.dma_start(out=out[ts(tt, 128), :], in_=o_sb[:])

            # ---- flash-style startup: feed ACT as early as possible ----
            # Scores-critical work is emitted (= prioritized) strictly ahead
            # of the V projections, which are pure PE filler in the ACT-paced
            # slack of rounds 1-2; pv(0,0) directly follows V.
            # round 0 (qg0, hp0): k chunk0 + q chunk0(tt0); scores chase the
            # k tt-groups as they land.
            qk_group(1, 0, 0)
            qk_group(0, 0, 0)
            p00 = p_alloc(0, 0)
            st_seg(0, 0, [0, 1], p00)
            qk_group(1, 0, 1)
            st_seg(0, 0, [2, 3], p00)
            qk_group(1, 0, 2)
            st_seg(0, 0, [4, 5], p00)
            qk_group(1, 0, 3)
            st_seg(0, 0, [6, 7], p00)

            # round 1 (qg0, hp1): k chunk1 + q chunk1(tt0)
            qk_group(1, 1, 0)
            qk_group(0, 1, 0)
            p01 = p_alloc(0, 1)
            st_seg(0, 1, [0, 1], p01)
            qk_group(1, 1, 1)
            st_seg(0, 1, [2, 3], p01)
            qk_group(1, 1, 2)
            st_seg(0, 1, [4, 5], p01)
            qk_group(1, 1, 3)
            st_seg(0, 1, [6, 7], p01)

            # round 2 (qg1, hp0) scores, then V in its ACT-slack
            qk_group(0, 0, 1)
            qk_group(0, 1, 1)
            st_part(1, 0)
            for tv in range(16):
                v_group(tv)

            # ---- pipelined main stream ----
            pv_part(0, 0)
            st_part(1, 1)
            pv_part(0, 1)
            qk_group(0, 0, 2)
            qk_group(0, 1, 2)
            st_part(2, 0)
            normalize_round(0, 0)
            pv_part(1, 0)
            st_part(2, 1)
            normalize_round(0, 1)
            outproj_chunk(0)
            pv_part(1, 1)
            qk_group(0, 0, 3)
            qk_group(0, 1, 3)
            st_part(3, 0)
            normalize_round(1, 0)
            pv_part(2, 0)
            st_part(3, 1)
            normalize_round(1, 1)
            outproj_chunk(1)
            pv_part(2, 1)
            normalize_round(2, 0)
            pv_part(3, 0)
            normalize_round(2, 1)
            outproj_chunk(2)
            pv_part(3, 1)
            normalize_round(3, 0)
            normalize_round(3, 1)
            outproj_chunk(3)

    nc.compile()
    return nc


def _get_nc():
    global _NC_CACHE
    if _NC_CACHE is None:
        _NC_CACHE = _build_nc()
    return _NC_CACHE


def kernel(x, w_qkv, b_qkv, w_out, b_out):
    global LAST_RESULT
    x = np.asarray(x, dtype=np.float32)
    w_qkv = np.asarray(w_qkv, dtype=np.float32)
    b_qkv = np.asarray(b_qkv, dtype=np.float32)
    w_out = np.asarray(w_out, dtype=np.float32)
    b_out = np.asarray(b_out, dtype=np.float32)

    bf = ml_dtypes.bfloat16

    def blk_w(w):  # [1024, n] -> [128, 8, n] (p, ci, n) contiguous
        n = w.shape[1]
        return np.ascontiguousarray(
            w.reshape(8, 128, n).transpose(1, 0, 2)
        ).astype(bf)

    in_maps = []
    for c in range(N_CORES):
        b, g = divmod(c, 4)
        cols = slice(CH * g, CH * (g + 1))
        bq = b_qkv[0 * C + CH * g : 0 * C + CH * (g + 1)]
        bk = b_qkv[1 * C + CH * g : 1 * C + CH * (g + 1)]
        # x^T token-blocked: [p, tt, ci, 512]
        xtb = np.ascontiguousarray(
            x[b].T.astype(bf).reshape(8, 128, 4, 512).transpose(1, 2, 0, 3)
        )
        # wout row-blocked: [p, hp, 1024]
        wob = np.ascontiguousarray(
            w_out[CH * g : CH * (g + 1), :].reshape(2, 128, C).transpose(1, 0, 2)
        ).astype(bf)
        in_maps.append(
            {
                "xt": xtb,
                "wq": blk_w(w_qkv[:, 0 * C :][:, cols]),
                "wk": blk_w(w_qkv[:, 1 * C :][:, cols]),
                "wv": blk_w(w_qkv[:, 2 * C :][:, cols]),
                "bqt": np.ascontiguousarray(bq.reshape(2, 128).T),
                "bkt": np.ascontiguousarray(bk.reshape(2, 128).T),
                "wout": wob,
            }
        )

    nc = _get_nc()
    LAST_RESULT = bass_utils.run_bass_kernel_spmd(
        nc, in_maps, core_ids=list(range(N_CORES))
    )

    full = np.zeros((B, T, C), dtype=np.float32)
    # bias folded once on the host: b_out plus the V-bias pushed through
    # w_out (normalized attention rows sum to 1, so bv contributes exactly
    # bv @ w_out to every token)
    full += b_out + b_qkv[2 * C : 3 * C] @ w_out
    for c in range(N_CORES):
        b = c // 4
        full[b] += LAST_RESULT.results[c]["out"]
    return full


# revision 21
# speedup vs baseline: 1.1563x; 1.1563x over previous
"""Multi-head self-attention on 8 Trainium2 NeuronCores (Bass/Tile).

Problem: x[2,2048,1024] -> MHA(16 heads, d_head 64) -> out[2,2048,1024].

Sharding (batch x head-group, Megatron-ish, collective-free):
  core c (0..7): batch b = c//4, head group g = c%4 (heads 4g..4g+3).
  Each core computes q/k/v projections for its 4 heads over its batch,
  attention for those heads, and a PARTIAL output projection
  attn_local[256ch] @ w_out[256ch rows] over the full sequence. The host
  sums the 4 partials per batch (the Megatron row-parallel all-reduce is
  folded into the unshard step; b_out and the V-bias term bv @ w_out are
  added once on the host -- exact, since softmax rows sum to 1).

On-core layout (TensorE compute in bf16, fp32 PSUM accumulation):
  - ACT (exp for softmax) is the bottleneck engine: 16.8M exps/core ~=
    147us of ACT instruction time. The schedule saturates ACT from the
    earliest possible point after the ~7us engine-boot preamble:
    * all inputs arrive in host-pre-blocked layouts that are contiguous
      per SBUF partition (cheap HWDGE triggers, 4-8KB DMA lines), split
      across the sync and scalar queues with x^T token-sliced;
    * k/q chunk-0 projections are emitted first so scores round 0 feeds
      ACT immediately; V projections and the remaining q/k chunks are
      pure PE filler behind ACT pacing.
  - exp p-tiles are paired per round ([128, 2heads, 8kp, 1024]) with 3
    rotating buffers = 3 rounds in flight, so round r+2's exps never
    wait on round r's PV consumption.
  - qT/kT in [channel, t] layout: scores^T = kT.T @ qT with the two
    heads of a chunk in partitions 0-63/64-127 -> concurrent K=64
    matmuls in disjoint PE row groups.
  - softmax: scores^T [128ki, qi] -> ACT exp (PSUM->SBUF bf16,
    scale=1/8 folded, no max subtraction: |s|/8 <= ~2).
  - PV: attn^T = V.T @ P~ as column-tiled concurrent M=64 head pairs;
    denominators via DVE bf16 add-tree + K=128 ones-matmul fold,
    reciprocal_approx_fast straight off PSUM; normalize one round
    behind PV (rep-matmul broadcast + DVE mul); out-projection +
    output DMA per query group.
"""

import numpy as np
import ml_dtypes

import concourse.bass as bass
import concourse.mybir as mybir
import concourse.tile as tile
from concourse import bacc
from concourse import bass_utils
from concourse.bass import ts

BF = mybir.dt.bfloat16
F32 = mybir.dt.float32

B, T, C = 2, 2048, 1024
H, DH = 16, 64
N_CORES = 8
HG = 4  # heads per core
CH = HG * DH  # 256 channels per core

LAST_RESULT = None  # BassKernelResults of the most recent run (for profiling)
_NC_CACHE = None


def _build_nc():
    nc = bacc.Bacc(
        "TRN2", target_bir_lowering=False, debug=False, num_devices=N_CORES
    )

    # host-pre-blocked layouts: every tensor is contiguous along its SBUF
    # partition's free dim, so each DMA is 128 x (one fat line).
    xt = nc.dram_tensor("xt", [128, 4, 8, 512], BF, kind="ExternalInput")
    wq = nc.dram_tensor("wq", [128, 8, CH], BF, kind="ExternalInput")
    wk = nc.dram_tensor("wk", [128, 8, CH], BF, kind="ExternalInput")
    wv = nc.dram_tensor("wv", [128, 8, CH], BF, kind="ExternalInput")
    bqt = nc.dram_tensor("bqt", [128, 2], F32, kind="ExternalInput")
    bkt = nc.dram_tensor("bkt", [128, 2], F32, kind="ExternalInput")
    wout = nc.dram_tensor("wout", [128, 2, C], BF, kind="ExternalInput")
    out = nc.dram_tensor("out", [T, C], F32, kind="ExternalOutput")

    with tile.TileContext(nc) as tc:
        with (
            tc.tile_pool(name="persist", bufs=1) as persist,
            tc.tile_pool(name="consts", bufs=1) as consts,
            tc.tile_pool(name="sbn", bufs=2) as sbn,
            tc.tile_pool(name="osb", bufs=3) as osb,
            tc.tile_pool(name="ps_st", bufs=2, space="PSUM") as ps_st,
            tc.tile_pool(name="ps_pv", bufs=2, space="PSUM") as ps_pv,
            tc.tile_pool(name="ps_misc", bufs=2, space="PSUM") as ps_misc,
        ):
            ones_bf = consts.tile([1, 128], BF)
            nc.vector.memset(ones_bf[:], 1.0)
            ones_col = consts.tile([128, 1], BF)
            nc.vector.memset(ones_col[:], 1.0)

            # xT is token-major: [p, tt(512-token block), ci, t-within-block]
            xT = persist.tile([128, 4, 8, 512], BF, tag="xT")
            wq_sb = persist.tile([128, 8, CH], BF, tag="wq")
            wk_sb = persist.tile([128, 8, CH], BF, tag="wk")
            wv_sb = persist.tile([128, 8, CH], BF, tag="wv")
            wout_sb = persist.tile([128, 2, C], BF, tag="wout")
            bqt_sb = consts.tile([128, 2], F32)
            bkt_sb = consts.tile([128, 2], F32)

            # ---- input DMA: sync + scalar HWDGE queues, critical-first ----
            # each queue's transfers share its engines, so the critical
            # tensors (xt0/wk/wq) must not share a queue with bulk loads.
            nc.scalar.dma_start(out=xT[:, 0], in_=xt[:, 0])
            nc.sync.dma_start(out=wk_sb[:], in_=wk[:])
            nc.scalar.dma_start(out=bkt_sb[:], in_=bkt[:])
            nc.scalar.dma_start(out=bqt_sb[:], in_=bqt[:])
            nc.scalar.dma_start(out=wq_sb[:], in_=wq[:])
            nc.sync.dma_start(out=xT[:, 1], in_=xt[:, 1])
            nc.sync.dma_start(out=xT[:, 2], in_=xt[:, 2])
            nc.scalar.dma_start(out=xT[:, 3], in_=xt[:, 3])
            nc.gpsimd.dma_start(out=wv_sb[:], in_=wv[:])
            nc.gpsimd.dma_start(out=wout_sb[:], in_=wout[:])

            # ---- PE warmup: ~6us of dummy matmuls during the DMA wait so
            # the HAM clock gate reaches 8/8 before the real projections.
            warm_src = consts.tile([128, 512], BF)
            nc.vector.memset(warm_src[:], 0.0)
            warm_ps = ps_misc.tile([128, 512], F32, tag="sm", name="warm")
            for i in range(14):
                nc.tensor.matmul(
                    warm_ps[0:1, :], warm_src[:, 0:1], warm_src[:],
                    start=(i == 0), stop=(i == 13),
                )

            # ---- persistent activations ----
            # qkT[:, 0:2, :] = qT chunks (hp), [:, 2:4, :] = kT chunks;
            # chunk hp rows 0-63 = head 2hp, rows 64-127 = head 2hp+1.
            qkT = persist.tile([128, 4, T], BF, tag="qkT")
            vext = persist.tile([128, T // 128, HG, DH], BF, tag="vext")
            attn_p = [
                [
                    persist.tile(
                        [128, 512], BF, tag=f"attnp{hp}_{qg}",
                        name=f"attnp{hp}_{qg}",
                    )
                    for qg in range(4)
                ]
                for hp in range(2)
            ]

            def qk_group(w_i, co, tt):
                """one [128,512] tile of qT (w_i=0) or kT (w_i=1), chunk co"""
                wsb = wq_sb if w_i == 0 else wk_sb
                bias_sb = bqt_sb if w_i == 0 else bkt_sb
                qp = ps_misc.tile([128, 512], F32, tag="sm", name="qp")
                for ci in range(8):
                    nc.tensor.matmul(
                        qp[:],
                        wsb[:, ci, ts(co, 128)],
                        xT[:, tt, ci, :],
                        start=(ci == 0),
                        stop=(ci == 7),
                    )
                # bias-add + cast on the DVE (keeps the ACT queue for exps)
                nc.vector.tensor_scalar_add(
                    qkT[:, 2 * w_i + co, ts(tt, 512)],
                    qp[:],
                    bias_sb[:, co : co + 1],
                )

            def v_group(tv):
                vp = ps_misc.tile([128, CH], F32, tag="sm", name="vp")
                for ci in range(8):
                    nc.tensor.matmul(
                        vp[:],
                        xT[:, tv // 4, ci, ts(tv % 4, 128)],
                        wv_sb[:, ci, :],
                        start=(ci == 0),
                        stop=(ci == 7),
                    )
                nc.vector.tensor_copy(
                    vext[:, tv, :, :],
                    vp[:].rearrange("p (h d) -> p h d", h=HG),
                )

            p_tiles = {}
            rec_tiles = {}
            tmp_tiles = {}

            def p_alloc(qg, hp):
                # paired tile: [p, head(A/B), kp, 1024]; one pool slot per
                # round -> bufs=3 keeps 3 rounds of exps live.
                p = osb.tile([128, 2, 8, 1024], BF, tag="p", bufs=3, name="p")
                p_tiles[(qg, hp)] = p
                return p

            def st_seg(qg, hp, kps, p):
                """scores^T + exp for head pair hp, query group qg, kp range."""
                qs = ts(qg, 512)
                for kp in kps:
                    stA = ps_st.tile([128, 1024], F32, tag="st", name="stA")
                    stB = ps_st.tile([128, 1024], F32, tag="st", name="stB")
                    for j in range(2):
                        ki = 2 * kp + j
                        nc.tensor.matmul(
                            stA[:, ts(j, 512)],
                            qkT[0:64, 2 + hp, ts(ki, 128)],
                            qkT[0:64, hp, qs],
                            start=True, stop=True,
                        )
                        nc.tensor.matmul(
                            stB[:, ts(j, 512)],
                            qkT[64:128, 2 + hp, ts(ki, 128)],
                            qkT[64:128, hp, qs],
                            start=True, stop=True,
                        )
                    nc.scalar.activation(
                        p[:, 0, kp, :], stA[:],
                        mybir.ActivationFunctionType.Exp, scale=1.0 / 8.0,
                    )
                    nc.scalar.activation(
                        p[:, 1, kp, :], stB[:],
                        mybir.ActivationFunctionType.Exp, scale=1.0 / 8.0,
                    )

            def st_part(qg, hp):
                p = p_alloc(qg, hp)
                st_seg(qg, hp, range(8), p)

            def pv_part(qg, hp):
                p = p_tiles.pop((qg, hp))
                # denominator add-trees first: they depend only on the exps,
                # so emitting them ahead of the PV matmuls keeps the DVE
                # queue from head-blocking on the PV-dependent tmp copy.
                t2_eng = nc.vector
                t4s = {}
                for hh in range(2):
                    t1 = sbn.tile([128, 4, 1024], BF, tag="t1", name="t1", bufs=1)
                    nc.vector.tensor_add(
                        t1[:], p[:, hh, 0:4, :], p[:, hh, 4:8, :]
                    )
                    t2 = sbn.tile([128, 2, 1024], BF, tag="t2", name="t2", bufs=1)
                    t2_eng.tensor_add(
                        t2[:], t1[:, 0:2, :], t1[:, 2:4, :]
                    )
                    t3 = sbn.tile([128, 1024], BF, tag="t3", name="t3", bufs=1)
                    nc.vector.tensor_add(
                        t3[:], t2[:, 0, :], t2[:, 1, :]
                    )
                    t4 = sbn.tile([128, 512], BF, tag="t4", name="t4", bufs=2)
                    nc.vector.tensor_add(
                        t4[:], t3[:, 0:512], t3[:, 512:1024]
                    )
                    t4s[hh] = t4
                # paired PV: head 2hp -> psum partitions 0-63 (col group 0-1),
                # head 2hp+1 -> partitions 64-127 (col group 2-3); the two
                # column-tiled matmul streams run concurrently on the PE.
                pv = ps_pv.tile([128, 512], F32, tag="pv", name="pv")
                for ki in range(16):
                    for hh in range(2):
                        h = 2 * hp + hh
                        nc.tensor.matmul(
                            pv[64 * hh : 64 * hh + 64, :],
                            vext[:, ki, h, :],
                            p[:, hh, ki // 2, ts(ki % 2, 512)],
                            start=(ki == 0),
                            stop=(ki == 15),
                        )
                # partition-axis fold of the partial denominators (K=128
                # ones-matmul), then the reciprocal chain (DVE-only)
                for hh in range(2):
                    h = 2 * hp + hh
                    dps = ps_misc.tile([128, 512], F32, tag="sm", name="dps")
                    nc.tensor.matmul(
                        dps[0:1, :], ones_col[:, 0:1], t4s[hh][:],
                        start=True, stop=True,
                    )
                    rec32 = sbn.tile([1, 512], F32, tag="rec32", name="rc", bufs=1)
                    nc.vector.reciprocal_approx_fast(out=rec32[:], in_=dps[0:1, :])
                    rec_bf = sbn.tile([1, 512], BF, tag="rec", name="rb", bufs=4)
                    nc.vector.tensor_copy(rec_bf[:], rec32[:])
                    rec_tiles[4 * qg + h] = rec_bf
                tmp = sbn.tile([128, 512], BF, tag="tmp", name="tmp", bufs=4)
                # final rounds' tmp copies on ScalarE: ACT is idle once the
                # exps end, and it keeps the tail off the backlogged DVE.
                tmp_eng = nc.scalar if qg == 3 else nc.vector
                if tmp_eng is nc.scalar:
                    nc.scalar.copy(tmp[:], pv[:])
                else:
                    nc.vector.tensor_copy(tmp[:], pv[:])
                tmp_tiles[(qg, hp)] = tmp

            def normalize_round(qg, hp):
                """rep-matmul + multiply -> attn_p[hp][qg] (both heads)."""
                rp = ps_misc.tile([128, 512], F32, tag="sm", name="rp")
                tmp = tmp_tiles.pop((qg, hp))
                for hh in range(2):
                    slot = 4 * qg + 2 * hp + hh
                    rows = slice(64 * hh, 64 * hh + 64)
                    nc.tensor.matmul(
                        rp[rows, :], ones_bf[0:1, 0:64], rec_tiles[slot][:],
                        start=True, stop=True,
                    )
                    nc.vector.tensor_mul(
                        attn_p[hp][qg][rows, :],
                        tmp[rows, :],
                        rp[rows, :],
                    )

            def outproj_chunk(qg):
                """partial out-projection rows for query group qg."""
                for tt4 in range(4):
                    tt = 4 * qg + tt4
                    o_sb = osb.tile([128, C], F32, tag="o", name="osb", bufs=1)
                    for cn in range(2):
                        op = ps_misc.tile(
                            [128, 512], F32, tag="sm", name="op"
                        )
                        for hp in range(2):
                            nc.tensor.matmul(
                                op[:],
                                attn_p[hp][qg][:, ts(tt4, 128)],
                                wout_sb[:, hp, ts(cn, 512)],
                                start=(hp == 0),
                                stop=(hp == 1),
                            )
                        if qg >= 2:
                            nc.scalar.copy(o_sb[:, ts(cn, 512)], op[:])
                        else:
                            nc.vector.tensor_copy(o_sb[:, ts(cn, 512)], op[:])
                    nc.sync.dma_start(out=out[ts(tt, 128), :], in_=o_sb[:])

            # ---- flash-style startup: feed ACT as early as possible ----
            # Scores-critical work is emitted (= prioritized) strictly ahead
            # of the V projections, which are pure PE filler in the ACT-paced
            # slack of rounds 1-2; pv(0,0) directly follows V.
            # round 0 (qg0, hp0): k chunk0 + q chunk0(tt0); scores chase the
            # k tt-groups as they land.
            qk_group(1, 0, 0)
            qk_group(0, 0, 0)
            p00 = p_alloc(0, 0)
            st_seg(0, 0, [0, 1], p00)
            qk_group(1, 0, 1)
            st_seg(0, 0, [2, 3], p00)
            qk_group(1, 0, 2)
            st_seg(0, 0, [4, 5], p00)
            qk_group(1, 0, 3)
            st_seg(0, 0, [6, 7], p00)

            # round 1 (qg0, hp1): k chunk1 + q chunk1(tt0)
            qk_group(1, 1, 0)
            qk_group(0, 1, 0)
            p01 = p_alloc(0, 1)
            st_seg(0, 1, [0, 1], p01)
            qk_group(1, 1, 1)
            st_seg(0, 1, [2, 3], p01)
            qk_group(1, 1, 2)
            st_seg(0, 1, [4, 5], p01)
            qk_group(1, 1, 3)
            st_seg(0, 1, [6, 7], p01)

            # round 2 (qg1, hp0) scores, then V in its ACT-slack
            qk_group(0, 0, 1)
            qk_group(0, 1, 1)
            st_part(1, 0)
            for tv in range(16):
                v_group(tv)

            # ---- pipelined main stream ----
            pv_part(0, 0)
            st_part(1, 1)
            pv_part(0, 1)
            qk_group(0, 0, 2)
            qk_group(0, 1, 2)
            st_part(2, 0)
            normalize_round(0, 0)
            pv_part(1, 0)
            st_part(2, 1)
            normalize_round(0, 1)
            outproj_chunk(0)
            pv_part(1, 1)
            qk_group(0, 0, 3)
            qk_group(0, 1, 3)
            st_part(3, 0)
            normalize_round(1, 0)
            pv_part(2, 0)
            st_part(3, 1)
            normalize_round(1, 1)
            outproj_chunk(1)
            pv_part(2, 1)
            normalize_round(2, 0)
            pv_part(3, 0)
            normalize_round(2, 1)
            outproj_chunk(2)
            pv_part(3, 1)
            normalize_round(3, 0)
            normalize_round(3, 1)
            outproj_chunk(3)

    nc.compile()
    return nc


def _get_nc():
    global _NC_CACHE
    if _NC_CACHE is None:
        _NC_CACHE = _build_nc()
    return _NC_CACHE


def kernel(x, w_qkv, b_qkv, w_out, b_out):
    global LAST_RESULT
    x = np.asarray(x, dtype=np.float32)
    w_qkv = np.asarray(w_qkv, dtype=np.float32)
    b_qkv = np.asarray(b_qkv, dtype=np.float32)
    w_out = np.asarray(w_out, dtype=np.float32)
    b_out = np.asarray(b_out, dtype=np.float32)

    bf = ml_dtypes.bfloat16

    def blk_w(w):  # [1024, n] -> [128, 8, n] (p, ci, n) contiguous
        n = w.shape[1]
        return np.ascontiguousarray(
            w.reshape(8, 128, n).transpose(1, 0, 2)
        ).astype(bf)

    in_maps = []
    for c in range(N_CORES):
        b, g = divmod(c, 4)
        cols = slice(CH * g, CH * (g + 1))
        bq = b_qkv[0 * C + CH * g : 0 * C + CH * (g + 1)]
        bk = b_qkv[1 * C + CH * g : 1 * C + CH * (g + 1)]
        # x^T token-blocked: [p, tt, ci, 512]
        xtb = np.ascontiguousarray(
            x[b].T.astype(bf).reshape(8, 128, 4, 512).transpose(1, 2, 0, 3)
        )
        # wout row-blocked: [p, hp, 1024]
        wob = np.ascontiguousarray(
            w_out[CH * g : CH * (g + 1), :].reshape(2, 128, C).transpose(1, 0, 2)
        ).astype(bf)
        in_maps.append(
            {
                "xt": xtb,
                "wq": blk_w(w_qkv[:, 0 * C :][:, cols]),
                "wk": blk_w(w_qkv[:, 1 * C :][:, cols]),
                "wv": blk_w(w_qkv[:, 2 * C :][:, cols]),
                "bqt": np.ascontiguousarray(bq.reshape(2, 128).T),
                "bkt": np.ascontiguousarray(bk.reshape(2, 128).T),
                "wout": wob,
            }
        )

    nc = _get_nc()
    LAST_RESULT = bass_utils.run_bass_kernel_spmd(
        nc, in_maps, core_ids=list(range(N_CORES))
    )

    full = np.zeros((B, T, C), dtype=np.float32)
    # bias folded once on the host: b_out plus the V-bias pushed through
    # w_out (normalized attention rows sum to 1, so bv contributes exactly
    # bv @ w_out to every token)
    full += b_out + b_qkv[2 * C : 3 * C] @ w_out
    for c in range(N_CORES):
        b = c // 4
        full[b] += LAST_RESULT.results[c]["out"]
    return full


# revision 25
# speedup vs baseline: 1.2135x; 1.0494x over previous
"""Multi-head self-attention on 8 Trainium2 NeuronCores (Bass/Tile).

Problem: x[2,2048,1024] -> MHA(16 heads, d_head 64) -> out[2,2048,1024].

Sharding (batch x head-group, Megatron-ish, collective-free):
  core c (0..7): batch b = c//4, head group g = c%4 (heads 4g..4g+3).
  Each core computes q/k/v projections for its 4 heads over its batch,
  attention for those heads, and a PARTIAL output projection
  attn_local[256ch] @ w_out[256ch rows] over the full sequence. The host
  sums the 4 partials per batch (the Megatron row-parallel all-reduce is
  folded into the unshard step; b_out and the V-bias term bv @ w_out are
  added once on the host -- exact, since softmax rows sum to 1).

On-core layout (TensorE compute in bf16, fp32 PSUM accumulation):
  - ACT (exp for softmax) is the bottleneck engine: 16.8M exps/core ~=
    147us of ACT instruction time. The schedule saturates ACT from the
    earliest possible point after the ~7us engine-boot preamble:
    * all inputs arrive in host-pre-blocked layouts that are contiguous
      per SBUF partition (cheap HWDGE triggers, 4-8KB DMA lines), split
      across the sync and scalar queues with x^T token-sliced;
    * k/q chunk-0 projections are emitted first so scores round 0 feeds
      ACT immediately; V projections and the remaining q/k chunks are
      pure PE filler behind ACT pacing.
  - exp p-tiles are paired per round ([128, 2heads, 8kp, 1024]) with 3
    rotating buffers = 3 rounds in flight, so round r+2's exps never
    wait on round r's PV consumption.
  - qT/kT in [channel, t] layout: scores^T = kT.T @ qT with the two
    heads of a chunk in partitions 0-63/64-127 -> concurrent K=64
    matmuls in disjoint PE row groups.
  - softmax: scores^T [128ki, qi] -> ACT exp (PSUM->SBUF bf16,
    scale=1/8 folded, no max subtraction: |s|/8 <= ~2).
  - PV: attn^T = V.T @ P~ as column-tiled concurrent M=64 head pairs;
    denominators via DVE bf16 add-tree + K=128 ones-matmul fold,
    reciprocal_approx_fast straight off PSUM; normalize one round
    behind PV (rep-matmul broadcast + DVE mul); out-projection +
    output DMA per query group.
"""

import numpy as np
import ml_dtypes

import concourse.bass as bass
import concourse.mybir as mybir
import concourse.tile as tile
from concourse import bacc
from concourse import bass_utils
from concourse.bass import ts

BF = mybir.dt.bfloat16
F32 = mybir.dt.float32

B, T, C = 2, 2048, 1024
H, DH = 16, 64
N_CORES = 8
HG = 4  # heads per core
CH = HG * DH  # 256 channels per core

LAST_RESULT = None  # BassKernelResults of the most recent run (for profiling)
_NC_CACHE = None


def _build_nc():
    nc = bacc.Bacc(
        "TRN2", target_bir_lowering=False, debug=False, num_devices=N_CORES
    )

    # host-pre-blocked layouts: every tensor is contiguous along its SBUF
    # partition's free dim, so each DMA is 128 x (one fat line).
    xt = nc.dram_tensor("xt", [128, 4, 8, 512], BF, kind="ExternalInput")
    wq = nc.dram_tensor("wq", [128, 8, CH], BF, kind="ExternalInput")
    wk = nc.dram_tensor("wk", [128, 8, CH], BF, kind="ExternalInput")
    wv = nc.dram_tensor("wv", [128, 8, CH], BF, kind="ExternalInput")
    bqt = nc.dram_tensor("bqt", [128, 2], F32, kind="ExternalInput")
    bkt = nc.dram_tensor("bkt", [128, 2], F32, kind="ExternalInput")
    wout = nc.dram_tensor("wout", [128, 2, C], BF, kind="ExternalInput")
    out = nc.dram_tensor("out", [T, C], BF, kind="ExternalOutput")

    with tile.TileContext(nc) as tc:
        with (
            tc.tile_pool(name="persist", bufs=1) as persist,
            tc.tile_pool(name="consts", bufs=1) as consts,
            tc.tile_pool(name="sbn", bufs=2) as sbn,
            tc.tile_pool(name="osb", bufs=3) as osb,
            tc.tile_pool(name="ps_st", bufs=2, space="PSUM") as ps_st,
            tc.tile_pool(name="ps_pv", bufs=2, space="PSUM") as ps_pv,
            tc.tile_pool(name="ps_misc", bufs=2, space="PSUM") as ps_misc,
        ):
            ones_bf = consts.tile([1, 128], BF)
            nc.vector.memset(ones_bf[:], 1.0)
            ones_col = consts.tile([128, 1], BF)
            nc.vector.memset(ones_col[:], 1.0)

            # xT is token-major: [p, tt(512-token block), ci, t-within-block]
            xT = persist.tile([128, 4, 8, 512], BF, tag="xT")
            wq_sb = persist.tile([128, 8, CH], BF, tag="wq")
            wk_sb = persist.tile([128, 8, CH], BF, tag="wk")
            wv_sb = persist.tile([128, 8, CH], BF, tag="wv")
            wout_sb = persist.tile([128, 2, C], BF, tag="wout")
            bqt_sb = consts.tile([128, 2], F32)
            bkt_sb = consts.tile([128, 2], F32)

            # ---- input DMA: 3-way queue split, critical-first ----
            # per-queue bandwidth is only ~130GB/s, so the critical tensors
            # (xt0 / wk / wq) go first on separate queues.
            nc.sync.dma_start(out=xT[:, 0], in_=xt[:, 0])
            nc.scalar.dma_start(out=wq_sb[:], in_=wq[:])
            nc.scalar.dma_start(out=wk_sb[:], in_=wk[:])
            nc.gpsimd.dma_start(out=bkt_sb[:], in_=bkt[:])
            nc.gpsimd.dma_start(out=bqt_sb[:], in_=bqt[:])
            nc.sync.dma_start(out=xT[:, 1], in_=xt[:, 1])
            nc.scalar.dma_start(out=xT[:, 2], in_=xt[:, 2])
            nc.sync.dma_start(out=xT[:, 3], in_=xt[:, 3])
            nc.gpsimd.dma_start(out=wv_sb[:], in_=wv[:])
            nc.gpsimd.dma_start(out=wout_sb[:], in_=wout[:])

            # ---- PE warmup: ~6us of dummy matmuls during the DMA wait so
            # the HAM clock gate reaches 8/8 before the real projections.
            warm_src = consts.tile([128, 512], BF)
            nc.vector.memset(warm_src[:], 0.0)
            warm_ps = ps_misc.tile([128, 512], F32, tag="sm", name="warm")
            for i in range(14):
                nc.tensor.matmul(
                    warm_ps[0:1, :], warm_src[:, 0:1], warm_src[:],
                    start=(i == 0), stop=(i == 13),
                )

            # ---- persistent activations ----
            # qkT[:, 0:2, :] = qT chunks (hp), [:, 2:4, :] = kT chunks;
            # chunk hp rows 0-63 = head 2hp, rows 64-127 = head 2hp+1.
            qkT = persist.tile([128, 4, T], BF, tag="qkT")
            vext = persist.tile([128, T // 128, HG, DH], BF, tag="vext")
            attn_p = [
                [
                    persist.tile(
                        [128, 512], BF, tag=f"attnp{hp}_{qg}",
                        name=f"attnp{hp}_{qg}",
                    )
                    for qg in range(4)
                ]
                for hp in range(2)
            ]

            def qk_group(w_i, co, tt):
                """one [128,512] tile of qT (w_i=0) or kT (w_i=1), chunk co"""
                wsb = wq_sb if w_i == 0 else wk_sb
                bias_sb = bqt_sb if w_i == 0 else bkt_sb
                qp = ps_misc.tile([128, 512], F32, tag="sm", name="qp")
                for ci in range(8):
                    nc.tensor.matmul(
                        qp[:],
                        wsb[:, ci, ts(co, 128)],
                        xT[:, tt, ci, :],
                        start=(ci == 0),
                        stop=(ci == 7),
                    )
                # bias-add + cast on the DVE (keeps the ACT queue for exps)
                nc.vector.tensor_scalar_add(
                    qkT[:, 2 * w_i + co, ts(tt, 512)],
                    qp[:],
                    bias_sb[:, co : co + 1],
                )

            def v_group(tv):
                vp = ps_misc.tile([128, CH], F32, tag="sm", name="vp")
                for ci in range(8):
                    nc.tensor.matmul(
                        vp[:],
                        xT[:, tv // 4, ci, ts(tv % 4, 128)],
                        wv_sb[:, ci, :],
                        start=(ci == 0),
                        stop=(ci == 7),
                    )
                nc.vector.tensor_copy(
                    vext[:, tv, :, :],
                    vp[:].rearrange("p (h d) -> p h d", h=HG),
                )

            p_tiles = {}
            rec_tiles = {}
            tmp_tiles = {}

            def p_alloc(qg, hp):
                # paired tile: [p, head(A/B), kp, 1024]; one pool slot per
                # round -> bufs=3 keeps 3 rounds of exps live.
                p = osb.tile([128, 2, 8, 1024], BF, tag="p", bufs=3, name="p")
                p_tiles[(qg, hp)] = p
                return p

            def st_seg(qg, hp, kps, p):
                """scores^T + exp for head pair hp, query group qg, kp range."""
                qs = ts(qg, 512)
                for kp in kps:
                    stA = ps_st.tile([128, 1024], F32, tag="st", name="stA")
                    stB = ps_st.tile([128, 1024], F32, tag="st", name="stB")
                    for j in range(2):
                        ki = 2 * kp + j
                        nc.tensor.matmul(
                            stA[:, ts(j, 512)],
                            qkT[0:64, 2 + hp, ts(ki, 128)],
                            qkT[0:64, hp, qs],
                            start=True, stop=True,
                        )
                        nc.tensor.matmul(
                            stB[:, ts(j, 512)],
                            qkT[64:128, 2 + hp, ts(ki, 128)],
                            qkT[64:128, hp, qs],
                            start=True, stop=True,
                        )
                    nc.scalar.activation(
                        p[:, 0, kp, :], stA[:],
                        mybir.ActivationFunctionType.Exp, scale=1.0 / 8.0,
                    )
                    nc.scalar.activation(
                        p[:, 1, kp, :], stB[:],
                        mybir.ActivationFunctionType.Exp, scale=1.0 / 8.0,
                    )

            def st_part(qg, hp):
                p = p_alloc(qg, hp)
                st_seg(qg, hp, range(8), p)

            def pv_part(qg, hp):
                p = p_tiles.pop((qg, hp))
                # denominator add-trees first: they depend only on the exps,
                # so emitting them ahead of the PV matmuls keeps the DVE
                # queue from head-blocking on the PV-dependent tmp copy.
                t2_eng = nc.vector
                t4s = {}
                for hh in range(2):
                    t1 = sbn.tile([128, 4, 1024], BF, tag="t1", name="t1", bufs=1)
                    nc.vector.tensor_add(
                        t1[:], p[:, hh, 0:4, :], p[:, hh, 4:8, :]
                    )
                    t2 = sbn.tile([128, 2, 1024], BF, tag="t2", name="t2", bufs=1)
                    t2_eng.tensor_add(
                        t2[:], t1[:, 0:2, :], t1[:, 2:4, :]
                    )
                    t3 = sbn.tile([128, 1024], BF, tag="t3", name="t3", bufs=1)
                    nc.vector.tensor_add(
                        t3[:], t2[:, 0, :], t2[:, 1, :]
                    )
                    t4 = sbn.tile([128, 512], BF, tag="t4", name="t4", bufs=2)
                    nc.vector.tensor_add(
                        t4[:], t3[:, 0:512], t3[:, 512:1024]
                    )
                    t4s[hh] = t4
                # paired PV: head 2hp -> psum partitions 0-63 (col group 0-1),
                # head 2hp+1 -> partitions 64-127 (col group 2-3); the two
                # column-tiled matmul streams run concurrently on the PE.
                pv = ps_pv.tile([128, 512], F32, tag="pv", name="pv")
                for ki in range(16):
                    for hh in range(2):
                        h = 2 * hp + hh
                        nc.tensor.matmul(
                            pv[64 * hh : 64 * hh + 64, :],
                            vext[:, ki, h, :],
                            p[:, hh, ki // 2, ts(ki % 2, 512)],
                            start=(ki == 0),
                            stop=(ki == 15),
                        )
                # partition-axis fold of the partial denominators (K=128
                # ones-matmul), then the reciprocal chain (DVE-only)
                for hh in range(2):
                    h = 2 * hp + hh
                    dps = ps_misc.tile([128, 512], F32, tag="sm", name="dps")
                    nc.tensor.matmul(
                        dps[0:1, :], ones_col[:, 0:1], t4s[hh][:],
                        start=True, stop=True,
                    )
                    rec32 = sbn.tile([1, 512], F32, tag="rec32", name="rc", bufs=1)
                    nc.vector.reciprocal_approx_fast(out=rec32[:], in_=dps[0:1, :])
                    rec_bf = sbn.tile([1, 512], BF, tag="rec", name="rb", bufs=4)
                    nc.vector.tensor_copy(rec_bf[:], rec32[:])
                    rec_tiles[4 * qg + h] = rec_bf
                tmp = sbn.tile([128, 512], BF, tag="tmp", name="tmp", bufs=4)
                # final rounds' tmp copies on ScalarE: ACT is idle once the
                # exps end, and it keeps the tail off the backlogged DVE.
                tmp_eng = nc.scalar if qg == 3 else nc.vector
                if tmp_eng is nc.scalar:
                    nc.scalar.copy(tmp[:], pv[:])
                else:
                    nc.vector.tensor_copy(tmp[:], pv[:])
                tmp_tiles[(qg, hp)] = tmp

            def normalize_round(qg, hp):
                """rep-matmul + multiply -> attn_p[hp][qg] (both heads)."""
                rp = ps_misc.tile([128, 512], F32, tag="sm", name="rp")
                tmp = tmp_tiles.pop((qg, hp))
                for hh in range(2):
                    slot = 4 * qg + 2 * hp + hh
                    rows = slice(64 * hh, 64 * hh + 64)
                    nc.tensor.matmul(
                        rp[rows, :], ones_bf[0:1, 0:64], rec_tiles[slot][:],
                        start=True, stop=True,
                    )
                    nc.vector.tensor_mul(
                        attn_p[hp][qg][rows, :],
                        tmp[rows, :],
                        rp[rows, :],
                    )

            def outproj_chunk(qg):
                """partial out-projection rows for query group qg."""
                for tt4 in range(4):
                    tt = 4 * qg + tt4
                    o_sb = osb.tile([128, C], BF, tag="o", name="osb", bufs=2)
                    for cn in range(2):
                        op = ps_misc.tile(
                            [128, 512], F32, tag="sm", name="op"
                        )
                        for hp in range(2):
                            nc.tensor.matmul(
                                op[:],
                                attn_p[hp][qg][:, ts(tt4, 128)],
                                wout_sb[:, hp, ts(cn, 512)],
                                start=(hp == 0),
                                stop=(hp == 1),
                            )
                        if qg >= 2:
                            nc.scalar.copy(o_sb[:, ts(cn, 512)], op[:])
                        else:
                            nc.vector.tensor_copy(o_sb[:, ts(cn, 512)], op[:])
                    # alternate output queues; bf16 halves the bytes so the
                    # last chunk's transfer is ~2us instead of ~4.4us.
                    oq = nc.sync if tt % 2 == 0 else nc.gpsimd
                    oq.dma_start(out=out[ts(tt, 128), :], in_=o_sb[:])

            # ---- flash-style startup: feed ACT as early as possible ----
            # Scores-critical work is emitted (= prioritized) strictly ahead
            # of the V projections, which are pure PE filler in the ACT-paced
            # slack of rounds 1-2; pv(0,0) directly follows V.
            # round 0 (qg0, hp0): k chunk0 + q chunk0(tt0); scores chase the
            # k tt-groups as they land.
            qk_group(1, 0, 0)
            qk_group(0, 0, 0)
            p00 = p_alloc(0, 0)
            st_seg(0, 0, [0, 1], p00)
            qk_group(1, 0, 1)
            st_seg(0, 0, [2, 3], p00)
            qk_group(1, 0, 2)
            st_seg(0, 0, [4, 5], p00)
            qk_group(1, 0, 3)
            st_seg(0, 0, [6, 7], p00)

            # round 1 (qg0, hp1): k chunk1 + q chunk1(tt0)
            qk_group(1, 1, 0)
            qk_group(0, 1, 0)
            p01 = p_alloc(0, 1)
            st_seg(0, 1, [0, 1], p01)
            qk_group(1, 1, 1)
            st_seg(0, 1, [2, 3], p01)
            qk_group(1, 1, 2)
            st_seg(0, 1, [4, 5], p01)
            qk_group(1, 1, 3)
            st_seg(0, 1, [6, 7], p01)

            # round 2 (qg1, hp0) scores, then V in its ACT-slack
            qk_group(0, 0, 1)
            qk_group(0, 1, 1)
            st_part(1, 0)
            for tv in range(16):
                v_group(tv)

            # ---- pipelined main stream ----
            pv_part(0, 0)
            st_part(1, 1)
            pv_part(0, 1)
            qk_group(0, 0, 2)
            qk_group(0, 1, 2)
            st_part(2, 0)
            normalize_round(0, 0)
            pv_part(1, 0)
            st_part(2, 1)
            normalize_round(0, 1)
            outproj_chunk(0)
            pv_part(1, 1)
            qk_group(0, 0, 3)
            qk_group(0, 1, 3)
            st_part(3, 0)
            normalize_round(1, 0)
            pv_part(2, 0)
            st_part(3, 1)
            normalize_round(1, 1)
            outproj_chunk(1)
            pv_part(2, 1)
            normalize_round(2, 0)
            pv_part(3, 0)
            normalize_round(2, 1)
            outproj_chunk(2)
            pv_part(3, 1)
            normalize_round(3, 0)
            normalize_round(3, 1)
            outproj_chunk(3)

    nc.compile()
    return nc


def _get_nc():
    global _NC_CACHE
    if _NC_CACHE is None:
        _NC_CACHE = _build_nc()
    return _NC_CACHE


def kernel(x, w_qkv, b_qkv, w_out, b_out):
    global LAST_RESULT
    x = np.asarray(x, dtype=np.float32)
    w_qkv = np.asarray(w_qkv, dtype=np.float32)
    b_qkv = np.asarray(b_qkv, dtype=np.float32)
    w_out = np.asarray(w_out, dtype=np.float32)
    b_out = np.asarray(b_out, dtype=np.float32)

    bf = ml_dtypes.bfloat16

    def blk_w(w):  # [1024, n] -> [128, 8, n] (p, ci, n) contiguous
        n = w.shape[1]
        return np.ascontiguousarray(
            w.reshape(8, 128, n).transpose(1, 0, 2)
        ).astype(bf)

    in_maps = []
    for c in range(N_CORES):
        b, g = divmod(c, 4)
        cols = slice(CH * g, CH * (g + 1))
        bq = b_qkv[0 * C + CH * g : 0 * C + CH * (g + 1)]
        bk = b_qkv[1 * C + CH * g : 1 * C + CH * (g + 1)]
        # x^T token-blocked: [p, tt, ci, 512]
        xtb = np.ascontiguousarray(
            x[b].T.astype(bf).reshape(8, 128, 4, 512).transpose(1, 2, 0, 3)
        )
        # wout row-blocked: [p, hp, 1024]
        wob = np.ascontiguousarray(
            w_out[CH * g : CH * (g + 1), :].reshape(2, 128, C).transpose(1, 0, 2)
        ).astype(bf)
        in_maps.append(
            {
                "xt": xtb,
                "wq": blk_w(w_qkv[:, 0 * C :][:, cols]),
                "wk": blk_w(w_qkv[:, 1 * C :][:, cols]),
                "wv": blk_w(w_qkv[:, 2 * C :][:, cols]),
                "bqt": np.ascontiguousarray(bq.reshape(2, 128).T),
                "bkt": np.ascontiguousarray(bk.reshape(2, 128).T),
                "wout": wob,
            }
        )

    nc = _get_nc()
    LAST_RESULT = bass_utils.run_bass_kernel_spmd(
        nc, in_maps, core_ids=list(range(N_CORES))
    )

    full = np.zeros((B, T, C), dtype=np.float32)
    # bias folded once on the host: b_out plus the V-bias pushed through
    # w_out (normalized attention rows sum to 1, so bv contributes exactly
    # bv @ w_out to every token)
    full += b_out + b_qkv[2 * C : 3 * C] @ w_out
    for c in range(N_CORES):
        b = c // 4
        full[b] += LAST_RESULT.results[c]["out"].astype(np.float32)
    return full


# revision 30
# speedup vs baseline: 1.2475x; 1.0280x over previous
"""Multi-head self-attention on 8 Trainium2 NeuronCores (Bass/Tile).

Problem: x[2,2048,1024] -> MHA(16 heads, d_head 64) -> out[2,2048,1024].

Sharding (batch x head-group, Megatron-ish, collective-free):
  core c (0..7): batch b = c//4, head group g = c%4 (heads 4g..4g+3).
  Each core computes q/k/v projections for its 4 heads over its batch,
  attention for those heads, and a PARTIAL output projection
  attn_local[256ch] @ w_out[256ch rows] over the full sequence. The host
  sums the 4 partials per batch (the Megatron row-parallel all-reduce is
  folded into the unshard step; b_out and the V-bias term bv @ w_out are
  added once on the host -- exact, since softmax rows sum to 1).

On-core layout (TensorE compute in bf16, fp32 PSUM accumulation):
  - ACT (exp for softmax) is the bottleneck engine: 16.8M exps/core ~=
    147us of ACT instruction time. The schedule saturates ACT from the
    earliest possible point after the ~7us engine-boot preamble:
    * all inputs arrive in host-pre-blocked layouts that are contiguous
      per SBUF partition (cheap HWDGE triggers, 4-8KB DMA lines), split
      across the sync and scalar queues with x^T token-sliced;
    * k/q chunk-0 projections are emitted first so scores round 0 feeds
      ACT immediately; V projections and the remaining q/k chunks are
      pure PE filler behind ACT pacing.
  - exp p-tiles are paired per round ([128, 2heads, 8kp, 1024]) with 3
    rotating buffers = 3 rounds in flight, so round r+2's exps never
    wait on round r's PV consumption.
  - qT/kT in [channel, t] layout: scores^T = kT.T @ qT with the two
    heads of a chunk in partitions 0-63/64-127 -> concurrent K=64
    matmuls in disjoint PE row groups.
  - softmax: scores^T [128ki, qi] -> ACT exp (PSUM->SBUF bf16,
    scale=1/8 folded, no max subtraction: |s|/8 <= ~2).
  - PV: attn^T = V.T @ P~ as column-tiled concurrent M=64 head pairs;
    denominators via DVE bf16 add-tree + K=128 ones-matmul fold,
    reciprocal_approx_fast straight off PSUM; normalize one round
    behind PV (rep-matmul broadcast + DVE mul); out-projection +
    output DMA per query group.
"""

import numpy as np
import ml_dtypes

import concourse.bass as bass
import concourse.mybir as mybir
import concourse.tile as tile
from concourse import bacc
from concourse import bass_utils
from concourse.bass import ts

BF = mybir.dt.bfloat16
F32 = mybir.dt.float32

B, T, C = 2, 2048, 1024
H, DH = 16, 64
N_CORES = 8
HG = 4  # heads per core
CH = HG * DH  # 256 channels per core

LAST_RESULT = None  # BassKernelResults of the most recent run (for profiling)
_NC_CACHE = None


def _build_nc():
    nc = bacc.Bacc(
        "TRN2", target_bir_lowering=False, debug=False, num_devices=N_CORES
    )

    # host-pre-blocked layouts: every tensor is contiguous along its SBUF
    # partition's free dim, so each DMA is 128 x (one fat line).
    xt = nc.dram_tensor("xt", [128, 4, 8, 512], BF, kind="ExternalInput")
    wq = nc.dram_tensor("wq", [128, 8, CH], BF, kind="ExternalInput")
    wk = nc.dram_tensor("wk", [128, 8, CH], BF, kind="ExternalInput")
    wv = nc.dram_tensor("wv", [128, 8, CH], BF, kind="ExternalInput")
    bqt = nc.dram_tensor("bqt", [128, 2], F32, kind="ExternalInput")
    bkt = nc.dram_tensor("bkt", [128, 2], F32, kind="ExternalInput")
    wout = nc.dram_tensor("wout", [128, 2, C], BF, kind="ExternalInput")
    out = nc.dram_tensor("out", [T, C], BF, kind="ExternalOutput")

    with tile.TileContext(nc) as tc:
        with (
            tc.tile_pool(name="persist", bufs=1) as persist,
            tc.tile_pool(name="consts", bufs=1) as consts,
            tc.tile_pool(name="sbn", bufs=2) as sbn,
            tc.tile_pool(name="osb", bufs=3) as osb,
            tc.tile_pool(name="ps_st", bufs=2, space="PSUM") as ps_st,
            tc.tile_pool(name="ps_pv", bufs=1, space="PSUM") as ps_pv,
            tc.tile_pool(name="ps_misc", bufs=3, space="PSUM") as ps_misc,
        ):
            ones_bf = consts.tile([1, 128], BF)
            nc.vector.memset(ones_bf[:], 1.0)
            ones_col = consts.tile([128, 1], BF)
            nc.vector.memset(ones_col[:], 1.0)

            # xT is token-major: [p, tt(512-token block), ci, t-within-block]
            xT = persist.tile([128, 4, 8, 512], BF, tag="xT")
            wq_sb = persist.tile([128, 8, CH], BF, tag="wq")
            wk_sb = persist.tile([128, 8, CH], BF, tag="wk")
            wv_sb = persist.tile([128, 8, CH], BF, tag="wv")
            wout_sb = persist.tile([128, 2, C], BF, tag="wout")
            bqt_sb = consts.tile([128, 2], F32)
            bkt_sb = consts.tile([128, 2], F32)

            # ---- input DMA: 3-way queue split, critical-first ----
            # per-queue bandwidth is only ~65-160GB/s, so the three critical
            # tensors (xt0 / wk / wq) go FIRST on three separate queues.
            nc.sync.dma_start(out=xT[:, 0], in_=xt[:, 0])
            nc.scalar.dma_start(out=wk_sb[:], in_=wk[:])
            nc.gpsimd.dma_start(out=bkt_sb[:], in_=bkt[:])
            nc.gpsimd.dma_start(out=bqt_sb[:], in_=bqt[:])
            nc.gpsimd.dma_start(out=wq_sb[:], in_=wq[:])
            nc.sync.dma_start(out=xT[:, 1], in_=xt[:, 1])
            nc.scalar.dma_start(out=xT[:, 2], in_=xt[:, 2])
            nc.sync.dma_start(out=xT[:, 3], in_=xt[:, 3])
            nc.gpsimd.dma_start(out=wv_sb[:], in_=wv[:])
            nc.gpsimd.dma_start(out=wout_sb[:], in_=wout[:])

            # ---- PE warmup: ~6us of dummy matmuls during the DMA wait so
            # the HAM clock gate reaches 8/8 before the real projections.
            warm_src = consts.tile([128, 512], BF)
            nc.vector.memset(warm_src[:], 0.0)
            warm_ps = ps_misc.tile([128, 512], F32, tag="sm", name="warm")
            for i in range(14):
                nc.tensor.matmul(
                    warm_ps[0:1, :], warm_src[:, 0:1], warm_src[:],
                    start=(i == 0), stop=(i == 13),
                )

            # ---- persistent activations ----
            # qkT[:, 0:2, :] = qT chunks (hp), [:, 2:4, :] = kT chunks;
            # chunk hp rows 0-63 = head 2hp, rows 64-127 = head 2hp+1.
            qkT = persist.tile([128, 4, T], BF, tag="qkT")
            vext = persist.tile([128, T // 128, HG, DH], BF, tag="vext")
            attn_p = [
                [
                    persist.tile(
                        [128, 512], BF, tag=f"attnp{hp}_{qg}",
                        name=f"attnp{hp}_{qg}",
                    )
                    for qg in range(4)
                ]
                for hp in range(2)
            ]

            def qk_group(w_i, co, tt):
                """one [128,512] tile of qT (w_i=0) or kT (w_i=1), chunk co"""
                wsb = wq_sb if w_i == 0 else wk_sb
                bias_sb = bqt_sb if w_i == 0 else bkt_sb
                qp = ps_misc.tile([128, 512], F32, tag="sm", name="qp")
                for ci in range(8):
                    nc.tensor.matmul(
                        qp[:],
                        wsb[:, ci, ts(co, 128)],
                        xT[:, tt, ci, :],
                        start=(ci == 0),
                        stop=(ci == 7),
                    )
                # bias-add + cast on the DVE (keeps the ACT queue for exps)
                nc.vector.tensor_scalar_add(
                    qkT[:, 2 * w_i + co, ts(tt, 512)],
                    qp[:],
                    bias_sb[:, co : co + 1],
                )

            def v_group(tv):
                vp = ps_misc.tile([128, CH], F32, tag="sm", name="vp")
                for ci in range(8):
                    nc.tensor.matmul(
                        vp[:],
                        xT[:, tv // 4, ci, ts(tv % 4, 128)],
                        wv_sb[:, ci, :],
                        start=(ci == 0),
                        stop=(ci == 7),
                    )
                nc.vector.tensor_copy(
                    vext[:, tv, :, :],
                    vp[:].rearrange("p (h d) -> p h d", h=HG),
                )

            p_tiles = {}
            rec_tiles = {}
            tmp_tiles = {}

            def p_alloc(qg, hp):
                # paired tile: [p, head(A/B), kp, 1024]; one pool slot per
                # round -> bufs=3 keeps 3 rounds of exps live.
                p = osb.tile([128, 2, 8, 1024], BF, tag="p", bufs=3, name="p")
                p_tiles[(qg, hp)] = p
                return p

            def st_seg(qg, hp, kps, p):
                """scores^T + exp for head pair hp, query group qg, kp range."""
                qs = ts(qg, 512)
                for kp in kps:
                    stA = ps_st.tile([128, 1024], F32, tag="st", name="stA")
                    stB = ps_st.tile([128, 1024], F32, tag="st", name="stB")
                    for j in range(2):
                        ki = 2 * kp + j
                        nc.tensor.matmul(
                            stA[:, ts(j, 512)],
                            qkT[0:64, 2 + hp, ts(ki, 128)],
                            qkT[0:64, hp, qs],
                            start=True, stop=True,
                        )
                        nc.tensor.matmul(
                            stB[:, ts(j, 512)],
                            qkT[64:128, 2 + hp, ts(ki, 128)],
                            qkT[64:128, hp, qs],
                            start=True, stop=True,
                        )
                    nc.scalar.activation(
                        p[:, 0, kp, :], stA[:],
                        mybir.ActivationFunctionType.Exp, scale=1.0 / 8.0,
                    )
                    nc.scalar.activation(
                        p[:, 1, kp, :], stB[:],
                        mybir.ActivationFunctionType.Exp, scale=1.0 / 8.0,
                    )

            def st_part(qg, hp):
                p = p_alloc(qg, hp)
                st_seg(qg, hp, range(8), p)

            def pv_part(qg, hp):
                p = p_tiles.pop((qg, hp))
                # denominator add-trees first: they depend only on the exps,
                # so emitting them ahead of the PV matmuls keeps the DVE
                # queue from head-blocking on the PV-dependent tmp copy.
                t2_eng = nc.vector
                t4s = {}
                for hh in range(2):
                    t1 = sbn.tile([128, 4, 1024], BF, tag="t1", name="t1", bufs=1)
                    nc.vector.tensor_add(
                        t1[:], p[:, hh, 0:4, :], p[:, hh, 4:8, :]
                    )
                    t2 = sbn.tile([128, 2, 1024], BF, tag="t2", name="t2", bufs=1)
                    t2_eng.tensor_add(
                        t2[:], t1[:, 0:2, :], t1[:, 2:4, :]
                    )
                    t3 = sbn.tile([128, 1024], BF, tag="t3", name="t3", bufs=1)
                    nc.vector.tensor_add(
                        t3[:], t2[:, 0, :], t2[:, 1, :]
                    )
                    t4 = sbn.tile([128, 512], BF, tag="t4", name="t4", bufs=2)
                    nc.vector.tensor_add(
                        t4[:], t3[:, 0:512], t3[:, 512:1024]
                    )
                    t4s[hh] = t4
                # paired PV: head 2hp -> psum partitions 0-63 (col group 0-1),
                # head 2hp+1 -> partitions 64-127 (col group 2-3); the two
                # column-tiled matmul streams run concurrently on the PE.
                pv = ps_pv.tile([128, 512], F32, tag="pv", name="pv")
                for ki in range(16):
                    for hh in range(2):
                        h = 2 * hp + hh
                        nc.tensor.matmul(
                            pv[64 * hh : 64 * hh + 64, :],
                            vext[:, ki, h, :],
                            p[:, hh, ki // 2, ts(ki % 2, 512)],
                            start=(ki == 0),
                            stop=(ki == 15),
                        )
                # partition-axis fold of the partial denominators (K=128
                # ones-matmul), then the reciprocal chain (DVE-only)
                for hh in range(2):
                    h = 2 * hp + hh
                    dps = ps_misc.tile([128, 512], F32, tag="sm", name="dps")
                    nc.tensor.matmul(
                        dps[0:1, :], ones_col[:, 0:1], t4s[hh][:],
                        start=True, stop=True,
                    )
                    rec32 = sbn.tile([1, 512], F32, tag="rec32", name="rc", bufs=1)
                    nc.vector.reciprocal_approx_fast(out=rec32[:], in_=dps[0:1, :])
                    rec_bf = sbn.tile([1, 512], BF, tag="rec", name="rb", bufs=4)
                    nc.vector.tensor_copy(rec_bf[:], rec32[:])
                    rec_tiles[4 * qg + h] = rec_bf
                tmp = sbn.tile([128, 512], BF, tag="tmp", name="tmp", bufs=3)
                # final rounds' tmp copies on ScalarE: ACT is idle once the
                # exps end, and it keeps the tail off the backlogged DVE.
                tmp_eng = nc.scalar if qg == 3 else nc.vector
                if tmp_eng is nc.scalar:
                    nc.scalar.copy(tmp[:], pv[:])
                else:
                    nc.vector.tensor_copy(tmp[:], pv[:])
                tmp_tiles[(qg, hp)] = tmp

            def normalize_round(qg, hp):
                """rep-matmul + multiply -> attn_p[hp][qg] (both heads)."""
                rp = ps_misc.tile([128, 512], F32, tag="sm", name="rp")
                tmp = tmp_tiles.pop((qg, hp))
                for hh in range(2):
                    slot = 4 * qg + 2 * hp + hh
                    rows = slice(64 * hh, 64 * hh + 64)
                    nc.tensor.matmul(
                        rp[rows, :], ones_bf[0:1, 0:64], rec_tiles[slot][:],
                        start=True, stop=True,
                    )
                    nc.vector.tensor_mul(
                        attn_p[hp][qg][rows, :],
                        tmp[rows, :],
                        rp[rows, :],
                    )

            def outproj_chunk(qg):
                """partial out-projection rows for query group qg."""
                for tt4 in range(4):
                    tt = 4 * qg + tt4
                    o_sb = osb.tile([128, C], BF, tag="o", name="osb", bufs=2)
                    for cn in range(2):
                        op = ps_misc.tile(
                            [128, 512], F32, tag="sm", name="op"
                        )
                        for hp in range(2):
                            nc.tensor.matmul(
                                op[:],
                                attn_p[hp][qg][:, ts(tt4, 128)],
                                wout_sb[:, hp, ts(cn, 512)],
                                start=(hp == 0),
                                stop=(hp == 1),
                            )
                        if qg >= 2:
                            nc.scalar.copy(o_sb[:, ts(cn, 512)], op[:])
                        else:
                            nc.vector.tensor_copy(o_sb[:, ts(cn, 512)], op[:])
                    # alternate output queues; bf16 halves the bytes so the
                    # last chunk's transfer is ~2us instead of ~4.4us.
                    oq = nc.sync if tt % 2 == 0 else nc.gpsimd
                    oq.dma_start(out=out[ts(tt, 128), :], in_=o_sb[:])

            # ---- flash-style startup: feed ACT as early as possible ----
            # Scores-critical work is emitted (= prioritized) strictly ahead
            # of the V projections, which are pure PE filler in the ACT-paced
            # slack of rounds 1-2; pv(0,0) directly follows V.
            # round 0 (qg0, hp0): k chunk0 + q chunk0(tt0); scores chase the
            # k tt-groups as they land.
            qk_group(1, 0, 0)
            qk_group(0, 0, 0)
            p00 = p_alloc(0, 0)
            st_seg(0, 0, [0, 1], p00)
            qk_group(1, 0, 1)
            st_seg(0, 0, [2, 3], p00)
            qk_group(1, 0, 2)
            st_seg(0, 0, [4, 5], p00)
            qk_group(1, 0, 3)
            st_seg(0, 0, [6, 7], p00)

            # round 1 (qg0, hp1): k chunk1 + q chunk1(tt0)
            qk_group(1, 1, 0)
            qk_group(0, 1, 0)
            p01 = p_alloc(0, 1)
            st_seg(0, 1, [0, 1], p01)
            qk_group(1, 1, 1)
            st_seg(0, 1, [2, 3], p01)
            qk_group(1, 1, 2)
            st_seg(0, 1, [4, 5], p01)
            qk_group(1, 1, 3)
            st_seg(0, 1, [6, 7], p01)

            # round 2 (qg1, hp0) scores, then V in its ACT-slack
            qk_group(0, 0, 1)
            qk_group(0, 1, 1)
            st_part(1, 0)
            for tv in range(16):
                v_group(tv)

            # ---- pipelined main stream ----
            pv_part(0, 0)
            st_part(1, 1)
            pv_part(0, 1)
            qk_group(0, 0, 2)
            qk_group(0, 1, 2)
            st_part(2, 0)
            normalize_round(0, 0)
            pv_part(1, 0)
            st_part(2, 1)
            normalize_round(0, 1)
            outproj_chunk(0)
            pv_part(1, 1)
            qk_group(0, 0, 3)
            qk_group(0, 1, 3)
            st_part(3, 0)
            normalize_round(1, 0)
            pv_part(2, 0)
            st_part(3, 1)
            normalize_round(1, 1)
            outproj_chunk(1)
            pv_part(2, 1)
            normalize_round(2, 0)
            pv_part(3, 0)
            normalize_round(2, 1)
            outproj_chunk(2)
            pv_part(3, 1)
            normalize_round(3, 0)
            normalize_round(3, 1)
            outproj_chunk(3)

    nc.compile()
    return nc


def _get_nc():
    global _NC_CACHE
    if _NC_CACHE is None:
        _NC_CACHE = _build_nc()
    return _NC_CACHE


def kernel(x, w_qkv, b_qkv, w_out, b_out):
    global LAST_RESULT
    x = np.asarray(x, dtype=np.float32)
    w_qkv = np.asarray(w_qkv, dtype=np.float32)
    b_qkv = np.asarray(b_qkv, dtype=np.float32)
    w_out = np.asarray(w_out, dtype=np.float32)
    b_out = np.asarray(b_out, dtype=np.float32)

    bf = ml_dtypes.bfloat16

    def blk_w(w):  # [1024, n] -> [128, 8, n] (p, ci, n) contiguous
        n = w.shape[1]
        return np.ascontiguousarray(
            w.reshape(8, 128, n).transpose(1, 0, 2)
        ).astype(bf)

    in_maps = []
    for c in range(N_CORES):
        b, g = divmod(c, 4)
        cols = slice(CH * g, CH * (g + 1))
        bq = b_qkv[0 * C + CH * g : 0 * C + CH * (g + 1)]
        bk = b_qkv[1 * C + CH * g : 1 * C + CH * (g + 1)]
        # x^T token-blocked: [p, tt, ci, 512]
        xtb = np.ascontiguousarray(
            x[b].T.astype(bf).reshape(8, 128, 4, 512).transpose(1, 2, 0, 3)
        )
        # wout row-blocked: [p, hp, 1024]
        wob = np.ascontiguousarray(
            w_out[CH * g : CH * (g + 1), :].reshape(2, 128, C).transpose(1, 0, 2)
        ).astype(bf)
        in_maps.append(
            {
                "xt": xtb,
                "wq": blk_w(w_qkv[:, 0 * C :][:, cols]),
                "wk": blk_w(w_qkv[:, 1 * C :][:, cols]),
                "wv": blk_w(w_qkv[:, 2 * C :][:, cols]),
                "bqt": np.ascontiguousarray(bq.reshape(2, 128).T),
                "bkt": np.ascontiguousarray(bk.reshape(2, 128).T),
                "wout": wob,
            }
        )

    nc = _get_nc()
    LAST_RESULT = bass_utils.run_bass_kernel_spmd(
        nc, in_maps, core_ids=list(range(N_CORES))
    )

    full = np.zeros((B, T, C), dtype=np.float32)
    # bias folded once on the host: b_out plus the V-bias pushed through
    # w_out (normalized attention rows sum to 1, so bv contributes exactly
    # bv @ w_out to every token)
    full += b_out + b_qkv[2 * C : 3 * C] @ w_out
    for c in range(N_CORES):
        b = c // 4
        full[b] += LAST_RESULT.results[c]["out"].astype(np.float32)
    return full


# revision 34
# speedup vs baseline: 1.3204x; 1.0584x over previous
"""Multi-head self-attention on 8 Trainium2 NeuronCores (Bass/Tile).

Problem: x[2,2048,1024] -> MHA(16 heads, d_head 64) -> out[2,2048,1024].

Sharding (batch x head-group, Megatron-ish, collective-free):
  core c (0..7): batch b = c//4, head group g = c%4 (heads 4g..4g+3).
  Each core computes q/k/v projections for its 4 heads over its batch,
  attention for those heads, and a PARTIAL output projection
  attn_local[256ch] @ w_out[256ch rows] over the full sequence. The host
  sums the 4 partials per batch (the Megatron row-parallel all-reduce is
  folded into the unshard step; b_out and the V-bias term bv @ w_out are
  added once on the host -- exact, since softmax rows sum to 1).

On-core layout (TensorE compute in bf16, fp32 PSUM accumulation):
  - ACT (exp for softmax) is the bottleneck engine: 16.8M exps/core ~=
    147us of ACT instruction time. The schedule saturates ACT from the
    earliest possible point after the ~7us engine-boot preamble:
    * all inputs arrive in host-pre-blocked layouts that are contiguous
      per SBUF partition (cheap HWDGE triggers, 4-8KB DMA lines), split
      across the sync and scalar queues with x^T token-sliced;
    * k/q chunk-0 projections are emitted first so scores round 0 feeds
      ACT immediately; V projections and the remaining q/k chunks are
      pure PE filler behind ACT pacing.
  - exp p-tiles are paired per round ([128, 2heads, 8kp, 1024]) with 3
    rotating buffers = 3 rounds in flight, so round r+2's exps never
    wait on round r's PV consumption.
  - qT/kT in [channel, t] layout: scores^T = kT.T @ qT with the two
    heads of a chunk in partitions 0-63/64-127 -> concurrent K=64
    matmuls in disjoint PE row groups.
  - softmax: scores^T [128ki, qi] -> ACT exp (PSUM->SBUF bf16,
    scale=1/8 folded, no max subtraction: |s|/8 <= ~2).
  - PV: attn^T = V.T @ P~ as column-tiled concurrent M=64 head pairs;
    denominators via DVE bf16 add-tree + K=128 ones-matmul fold,
    reciprocal_approx_fast straight off PSUM; normalize one round
    behind PV (rep-matmul broadcast + DVE mul); out-projection +
    output DMA per query group.
"""

import numpy as np
import ml_dtypes

import concourse.bass as bass
import concourse.mybir as mybir
import concourse.tile as tile
from concourse import bacc
from concourse import bass_utils
from concourse.bass import ts

BF = mybir.dt.bfloat16
F32 = mybir.dt.float32

B, T, C = 2, 2048, 1024
H, DH = 16, 64
N_CORES = 8
HG = 4  # heads per core
CH = HG * DH  # 256 channels per core

LAST_RESULT = None  # BassKernelResults of the most recent run (for profiling)
_NC_CACHE = None


def _build_nc():
    nc = bacc.Bacc(
        "TRN2", target_bir_lowering=False, debug=False, num_devices=N_CORES
    )

    # host-pre-blocked layouts: every tensor is contiguous along its SBUF
    # partition's free dim, so each DMA is 128 x (one fat line).
    F8 = mybir.dt.float8e4
    xt = nc.dram_tensor("xt", [128, 4, 8, 512], BF, kind="ExternalInput")
    xq8 = nc.dram_tensor("xq8", [128, 4, 4, 2, 512], F8, kind="ExternalInput")
    wq = nc.dram_tensor("wq", [128, 4, 2, CH], F8, kind="ExternalInput")
    wk = nc.dram_tensor("wk", [128, 4, 2, CH], F8, kind="ExternalInput")
    wv = nc.dram_tensor("wv", [128, 8, CH], BF, kind="ExternalInput")
    bqt = nc.dram_tensor("bqt", [128, 2], F32, kind="ExternalInput")
    bkt = nc.dram_tensor("bkt", [128, 2], F32, kind="ExternalInput")
    wout = nc.dram_tensor("wout", [128, 2, C], BF, kind="ExternalInput")
    out = nc.dram_tensor("out", [T, C], BF, kind="ExternalOutput")

    with tile.TileContext(nc) as tc:
        with (
            tc.tile_pool(name="persist", bufs=1) as persist,
            tc.tile_pool(name="consts", bufs=1) as consts,
            tc.tile_pool(name="sbn", bufs=2) as sbn,
            tc.tile_pool(name="osb", bufs=3) as osb,
            tc.tile_pool(name="ps_st", bufs=2, space="PSUM") as ps_st,
            tc.tile_pool(name="ps_pv", bufs=1, space="PSUM") as ps_pv,
            tc.tile_pool(name="ps_misc", bufs=3, space="PSUM") as ps_misc,
        ):
            ones_bf = consts.tile([1, 128], BF)
            nc.vector.memset(ones_bf[:], 1.0)
            ones_col = consts.tile([128, 1], BF)
            nc.vector.memset(ones_col[:], 1.0)

            # fp8 x (DoubleRow-packed) for q/k; bf16 x arrives per-tt into
            # a rotating buffer for the V projections.
            x8_sb = persist.tile([128, 4, 4, 2, 512], F8, tag="x8")
            wq_sb = persist.tile([128, 4, 2, CH], F8, tag="wq")
            wk_sb = persist.tile([128, 4, 2, CH], F8, tag="wk")
            wv_sb = persist.tile([128, 8, CH], BF, tag="wv")
            wout_sb = persist.tile([128, 2, C], BF, tag="wout")
            bqt_sb = consts.tile([128, 2], F32)
            bkt_sb = consts.tile([128, 2], F32)

            # ---- input DMA: 3-way queue split, critical-first ----
            # per-queue bandwidth is only ~65-160GB/s, so the critical
            # tensors (x8-tt0 / wk / wq, all small now) lead their queues.
            nc.sync.dma_start(out=x8_sb[:, 0], in_=xq8[:, 0])
            nc.scalar.dma_start(out=wk_sb[:], in_=wk[:])
            nc.scalar.dma_start(out=bkt_sb[:], in_=bkt[:])
            nc.scalar.dma_start(out=bqt_sb[:], in_=bqt[:])
            nc.scalar.dma_start(out=wq_sb[:], in_=wq[:])
            nc.sync.dma_start(out=x8_sb[:, 1], in_=xq8[:, 1])
            nc.sync.dma_start(out=x8_sb[:, 2], in_=xq8[:, 2])
            nc.sync.dma_start(out=x8_sb[:, 3], in_=xq8[:, 3])
            # bulk loads ride the gpsimd SWDGE queue: its slow descriptor
            # generation naturally defers them past the critical fp8/weight
            # loads (all queues share ~330GB/s of HBM bandwidth).
            nc.gpsimd.dma_start(out=wv_sb[:], in_=wv[:])
            xv_tiles = []
            for vtt in range(4):
                xv = osb.tile([128, 8, 512], BF, tag="xv", name="xv", bufs=2)
                nc.gpsimd.dma_start(out=xv[:], in_=xt[:, vtt])
                xv_tiles.append(xv)
            nc.gpsimd.dma_start(out=wout_sb[:], in_=wout[:])

            # ---- PE warmup: ~6us of dummy matmuls during the DMA wait so
            # the HAM clock gate reaches 8/8 before the real projections.
            warm_src = consts.tile([128, 512], BF)
            nc.vector.memset(warm_src[:], 0.0)
            warm_ps = ps_misc.tile([128, 512], F32, tag="sm", name="warm")
            for i in range(14):
                nc.tensor.matmul(
                    warm_ps[0:1, :], warm_src[:, 0:1], warm_src[:],
                    start=(i == 0), stop=(i == 13),
                )

            # ---- persistent activations ----
            # qkT[:, 0:2, :] = qT chunks (hp), [:, 2:4, :] = kT chunks;
            # chunk hp rows 0-63 = head 2hp, rows 64-127 = head 2hp+1.
            qkT = persist.tile([128, 4, T], BF, tag="qkT")
            vext = persist.tile([128, T // 128, HG, DH], BF, tag="vext")
            attn_p = [
                [
                    persist.tile(
                        [128, 512], BF, tag=f"attnp{hp}_{qg}",
                        name=f"attnp{hp}_{qg}",
                    )
                    for qg in range(4)
                ]
                for hp in range(2)
            ]

            def qk_group(w_i, co, tt):
                """one [128,512] tile of qT (w_i=0) or kT (w_i=1), chunk co"""
                wsb = wq_sb if w_i == 0 else wk_sb
                bias_sb = bqt_sb if w_i == 0 else bkt_sb
                qp = ps_misc.tile([128, 512], F32, tag="sm", name="qp")
                for jp in range(4):
                    nc.tensor.matmul(
                        qp[:],
                        wsb[:, jp, :, ts(co, 128)],
                        x8_sb[:, tt, jp, :, :],
                        perf_mode=mybir.MatmulPerfMode.DoubleRow,
                        start=(jp == 0),
                        stop=(jp == 3),
                    )
                # bias-add + cast on the DVE (keeps the ACT queue for exps)
                nc.vector.tensor_scalar_add(
                    qkT[:, 2 * w_i + co, ts(tt, 512)],
                    qp[:],
                    bias_sb[:, co : co + 1],
                )

            def v_group(tv):
                vp = ps_misc.tile([128, CH], F32, tag="sm", name="vp")
                for ci in range(8):
                    nc.tensor.matmul(
                        vp[:],
                        xv_tiles[tv // 4][:, ci, ts(tv % 4, 128)],
                        wv_sb[:, ci, :],
                        start=(ci == 0),
                        stop=(ci == 7),
                    )
                nc.vector.tensor_copy(
                    vext[:, tv, :, :],
                    vp[:].rearrange("p (h d) -> p h d", h=HG),
                )

            p_tiles = {}
            rec_tiles = {}
            tmp_tiles = {}

            def p_alloc(qg, hp):
                # paired tile: [p, head(A/B), kp, 1024]; one pool slot per
                # round -> bufs=3 keeps 3 rounds of exps live.
                p = osb.tile([128, 2, 8, 1024], BF, tag="p", bufs=3, name="p")
                p_tiles[(qg, hp)] = p
                return p

            def st_seg(qg, hp, kps, p):
                """scores^T + exp for head pair hp, query group qg, kp range."""
                qs = ts(qg, 512)
                for kp in kps:
                    stA = ps_st.tile([128, 1024], F32, tag="st", name="stA")
                    stB = ps_st.tile([128, 1024], F32, tag="st", name="stB")
                    for j in range(2):
                        ki = 2 * kp + j
                        nc.tensor.matmul(
                            stA[:, ts(j, 512)],
                            qkT[0:64, 2 + hp, ts(ki, 128)],
                            qkT[0:64, hp, qs],
                            start=True, stop=True,
                        )
                        nc.tensor.matmul(
                            stB[:, ts(j, 512)],
                            qkT[64:128, 2 + hp, ts(ki, 128)],
                            qkT[64:128, hp, qs],
                            start=True, stop=True,
                        )
                    nc.scalar.activation(
                        p[:, 0, kp, :], stA[:],
                        mybir.ActivationFunctionType.Exp, scale=1.0 / 8.0,
                    )
                    nc.scalar.activation(
                        p[:, 1, kp, :], stB[:],
                        mybir.ActivationFunctionType.Exp, scale=1.0 / 8.0,
                    )

            def st_part(qg, hp):
                p = p_alloc(qg, hp)
                st_seg(qg, hp, range(8), p)

            def pv_part(qg, hp):
                p = p_tiles.pop((qg, hp))
                # denominator add-trees first: they depend only on the exps,
                # so emitting them ahead of the PV matmuls keeps the DVE
                # queue from head-blocking on the PV-dependent tmp copy.
                t2_eng = nc.vector
                t4s = {}
                for hh in range(2):
                    t1 = sbn.tile([128, 4, 1024], BF, tag="t1", name="t1", bufs=1)
                    nc.vector.tensor_add(
                        t1[:], p[:, hh, 0:4, :], p[:, hh, 4:8, :]
                    )
                    t2 = sbn.tile([128, 2, 1024], BF, tag="t2", name="t2", bufs=1)
                    t2_eng.tensor_add(
                        t2[:], t1[:, 0:2, :], t1[:, 2:4, :]
                    )
                    t3 = sbn.tile([128, 1024], BF, tag="t3", name="t3", bufs=1)
                    nc.vector.tensor_add(
                        t3[:], t2[:, 0, :], t2[:, 1, :]
                    )
                    t4 = sbn.tile([128, 512], BF, tag="t4", name="t4", bufs=2)
                    nc.vector.tensor_add(
                        t4[:], t3[:, 0:512], t3[:, 512:1024]
                    )
                    t4s[hh] = t4
                # paired PV: head 2hp -> psum partitions 0-63 (col group 0-1),
                # head 2hp+1 -> partitions 64-127 (col group 2-3); the two
                # column-tiled matmul streams run concurrently on the PE.
                pv = ps_pv.tile([128, 512], F32, tag="pv", name="pv")
                for ki in range(16):
                    for hh in range(2):
                        h = 2 * hp + hh
                        nc.tensor.matmul(
                            pv[64 * hh : 64 * hh + 64, :],
                            vext[:, ki, h, :],
                            p[:, hh, ki // 2, ts(ki % 2, 512)],
                            start=(ki == 0),
                            stop=(ki == 15),
                        )
                # partition-axis fold of the partial denominators (K=128
                # ones-matmul), then the reciprocal chain (DVE-only)
                recs = []
                for hh in range(2):
                    dps = ps_misc.tile([128, 512], F32, tag="sm", name="dps")
                    nc.tensor.matmul(
                        dps[0:1, :], ones_col[:, 0:1], t4s[hh][:],
                        start=True, stop=True,
                    )
                    rec32 = sbn.tile([1, 512], F32, tag="rec32", name="rc", bufs=1)
                    nc.vector.reciprocal_approx_fast(out=rec32[:], in_=dps[0:1, :])
                    rec_bf = sbn.tile([1, 512], BF, tag="rec", name="rb", bufs=4)
                    nc.vector.tensor_copy(rec_bf[:], rec32[:])
                    recs.append(rec_bf)
                rec_tiles[(qg, hp)] = recs
                tmp = sbn.tile([128, 512], BF, tag="tmp", name="tmp", bufs=3)
                # final rounds' tmp copies on ScalarE: ACT is idle once the
                # exps end, and it keeps the tail off the backlogged DVE.
                tmp_eng = nc.scalar if qg == 3 else nc.vector
                if tmp_eng is nc.scalar:
                    nc.scalar.copy(tmp[:], pv[:])
                else:
                    nc.vector.tensor_copy(tmp[:], pv[:])
                tmp_tiles[(qg, hp)] = tmp

            def normalize_round(qg, hp):
                """rep-matmul + multiply -> attn_p[hp][qg] (both heads)."""
                rp = ps_misc.tile([128, 512], F32, tag="sm", name="rp")
                tmp = tmp_tiles.pop((qg, hp))
                recs = rec_tiles.pop((qg, hp))
                for hh in range(2):
                    rows = slice(64 * hh, 64 * hh + 64)
                    nc.tensor.matmul(
                        rp[rows, :], ones_bf[0:1, 0:64], recs[hh][:],
                        start=True, stop=True,
                    )
                nc.vector.tensor_mul(
                    attn_p[hp][qg][:], tmp[:], rp[:],
                )

            def outproj_chunk(qg):
                """partial out-projection rows for query group qg."""
                for tt4 in range(4):
                    tt = 4 * qg + tt4
                    o_sb = osb.tile([128, C], BF, tag="o", name="osb", bufs=2)
                    for cn in range(2):
                        op = ps_misc.tile(
                            [128, 512], F32, tag="sm", name="op"
                        )
                        for hp in range(2):
                            nc.tensor.matmul(
                                op[:],
                                attn_p[hp][qg][:, ts(tt4, 128)],
                                wout_sb[:, hp, ts(cn, 512)],
                                start=(hp == 0),
                                stop=(hp == 1),
                            )
                        if qg >= 2:
                            nc.scalar.copy(o_sb[:, ts(cn, 512)], op[:])
                        else:
                            nc.vector.tensor_copy(o_sb[:, ts(cn, 512)], op[:])
                    # alternate output queues; bf16 halves the bytes so the
                    # last chunk's transfer is ~2us instead of ~4.4us.
                    oq = nc.sync if tt % 2 == 0 else nc.gpsimd
                    oq.dma_start(out=out[ts(tt, 128), :], in_=o_sb[:])

            # ---- flash-style startup: feed ACT as early as possible ----
            # Scores-critical work is emitted (= prioritized) strictly ahead
            # of the V projections, which are pure PE filler in the ACT-paced
            # slack of rounds 1-2; pv(0,0) directly follows V.
            # round 0 (qg0, hp0): k chunk0 + q chunk0(tt0); scores chase the
            # k tt-groups as they land.
            qk_group(1, 0, 0)
            qk_group(0, 0, 0)
            p00 = p_alloc(0, 0)
            st_seg(0, 0, [0, 1], p00)
            qk_group(1, 0, 1)
            st_seg(0, 0, [2, 3], p00)
            qk_group(1, 0, 2)
            st_seg(0, 0, [4, 5], p00)
            qk_group(1, 0, 3)
            st_seg(0, 0, [6, 7], p00)

            # round 1 (qg0, hp1): k chunk1 + q chunk1(tt0)
            qk_group(1, 1, 0)
            qk_group(0, 1, 0)
            p01 = p_alloc(0, 1)
            st_seg(0, 1, [0, 1], p01)
            qk_group(1, 1, 1)
            st_seg(0, 1, [2, 3], p01)
            qk_group(1, 1, 2)
            st_seg(0, 1, [4, 5], p01)
            qk_group(1, 1, 3)
            st_seg(0, 1, [6, 7], p01)

            # round 2 (qg1, hp0) scores, then V in its ACT-slack
            qk_group(0, 0, 1)
            qk_group(0, 1, 1)
            st_part(1, 0)
            for tv in range(16):
                v_group(tv)

            # ---- pipelined main stream ----
            pv_part(0, 0)
            st_part(1, 1)
            pv_part(0, 1)
            qk_group(0, 0, 2)
            qk_group(0, 1, 2)
            st_part(2, 0)
            normalize_round(0, 0)
            pv_part(1, 0)
            st_part(2, 1)
            normalize_round(0, 1)
            outproj_chunk(0)
            pv_part(1, 1)
            qk_group(0, 0, 3)
            qk_group(0, 1, 3)
            st_part(3, 0)
            normalize_round(1, 0)
            pv_part(2, 0)
            st_part(3, 1)
            normalize_round(1, 1)
            outproj_chunk(1)
            pv_part(2, 1)
            normalize_round(2, 0)
            pv_part(3, 0)
            normalize_round(2, 1)
            outproj_chunk(2)
            pv_part(3, 1)
            normalize_round(3, 0)
            normalize_round(3, 1)
            outproj_chunk(3)

    nc.compile()
    return nc


def _get_nc():
    global _NC_CACHE
    if _NC_CACHE is None:
        _NC_CACHE = _build_nc()
    return _NC_CACHE


def kernel(x, w_qkv, b_qkv, w_out, b_out):
    global LAST_RESULT
    x = np.asarray(x, dtype=np.float32)
    w_qkv = np.asarray(w_qkv, dtype=np.float32)
    b_qkv = np.asarray(b_qkv, dtype=np.float32)
    w_out = np.asarray(w_out, dtype=np.float32)
    b_out = np.asarray(b_out, dtype=np.float32)

    bf = ml_dtypes.bfloat16

    f8 = ml_dtypes.float8_e4m3fn

    def blk_w(w):  # [1024, n] -> [128, 8, n] (p, ci, n) contiguous
        n = w.shape[1]
        return np.ascontiguousarray(
            w.reshape(8, 128, n).transpose(1, 0, 2)
        ).astype(bf)

    def blk_w8(w):  # [1024, n] -> [128, 4, 2, n] DoubleRow-packed fp8
        n = w.shape[1]
        return np.ascontiguousarray(
            w.reshape(4, 2, 128, n).transpose(2, 0, 1, 3).astype(f8)
        )

    in_maps = []
    for c in range(N_CORES):
        b, g = divmod(c, 4)
        cols = slice(CH * g, CH * (g + 1))
        bq = b_qkv[0 * C + CH * g : 0 * C + CH * (g + 1)]
        bk = b_qkv[1 * C + CH * g : 1 * C + CH * (g + 1)]
        # x^T token-blocked: [p, tt, ci, 512]
        xtb = np.ascontiguousarray(
            x[b].T.astype(bf).reshape(8, 128, 4, 512).transpose(1, 2, 0, 3)
        )
        # fp8 x^T DoubleRow-packed: [p, tt, jp, ko, 512]
        x8b = np.ascontiguousarray(
            x[b].T.reshape(4, 2, 128, 4, 512).transpose(2, 3, 0, 1, 4).astype(f8)
        )
        # wout row-blocked: [p, hp, 1024]
        wob = np.ascontiguousarray(
            w_out[CH * g : CH * (g + 1), :].reshape(2, 128, C).transpose(1, 0, 2)
        ).astype(bf)
        in_maps.append(
            {
                "xt": xtb,
                "xq8": x8b,
                "wq": blk_w8(w_qkv[:, 0 * C :][:, cols]),
                "wk": blk_w8(w_qkv[:, 1 * C :][:, cols]),
                "wv": blk_w(w_qkv[:, 2 * C :][:, cols]),
                "bqt": np.ascontiguousarray(bq.reshape(2, 128).T),
                "bkt": np.ascontiguousarray(bk.reshape(2, 128).T),
                "wout": wob,
            }
        )

    nc = _get_nc()
    LAST_RESULT = bass_utils.run_bass_kernel_spmd(
        nc, in_maps, core_ids=list(range(N_CORES))
    )

    full = np.zeros((B, T, C), dtype=np.float32)
    # bias folded once on the host: b_out plus the V-bias pushed through
    # w_out (normalized attention rows sum to 1, so bv contributes exactly
    # bv @ w_out to every token)
    full += b_out + b_qkv[2 * C : 3 * C] @ w_out
    for c in range(N_CORES):
        b = c // 4
        full[b] += LAST_RESULT.results[c]["out"].astype(np.float32)
    return full


# revision 35
# speedup vs baseline: 1.3247x; 1.0033x over previous
"""Multi-head self-attention on 8 Trainium2 NeuronCores (Bass/Tile).

Problem: x[2,2048,1024] -> MHA(16 heads, d_head 64) -> out[2,2048,1024].

Sharding (batch x head-group, Megatron-ish, collective-free):
  core c (0..7): batch b = c//4, head group g = c%4 (heads 4g..4g+3).
  Each core computes q/k/v projections for its 4 heads over its batch,
  attention for those heads, and a PARTIAL output projection
  attn_local[256ch] @ w_out[256ch rows] over the full sequence. The host
  sums the 4 partials per batch (the Megatron row-parallel all-reduce is
  folded into the unshard step; b_out and the V-bias term bv @ w_out are
  added once on the host -- exact, since softmax rows sum to 1).

On-core layout (TensorE compute in bf16, fp32 PSUM accumulation):
  - ACT (exp for softmax) is the bottleneck engine: 16.8M exps/core ~=
    147us of ACT instruction time. The schedule saturates ACT from the
    earliest possible point after the ~7us engine-boot preamble:
    * all inputs arrive in host-pre-blocked layouts that are contiguous
      per SBUF partition (cheap HWDGE triggers, 4-8KB DMA lines), split
      across the sync and scalar queues with x^T token-sliced;
    * k/q chunk-0 projections are emitted first so scores round 0 feeds
      ACT immediately; V projections and the remaining q/k chunks are
      pure PE filler behind ACT pacing.
  - exp p-tiles are paired per round ([128, 2heads, 8kp, 1024]) with 3
    rotating buffers = 3 rounds in flight, so round r+2's exps never
    wait on round r's PV consumption.
  - qT/kT in [channel, t] layout: scores^T = kT.T @ qT with the two
    heads of a chunk in partitions 0-63/64-127 -> concurrent K=64
    matmuls in disjoint PE row groups.
  - softmax: scores^T [128ki, qi] -> ACT exp (PSUM->SBUF bf16,
    scale=1/8 folded, no max subtraction: |s|/8 <= ~2).
  - PV: attn^T = V.T @ P~ as column-tiled concurrent M=64 head pairs;
    denominators via DVE bf16 add-tree + K=128 ones-matmul fold,
    reciprocal_approx_fast straight off PSUM; normalize one round
    behind PV (rep-matmul broadcast + DVE mul); out-projection +
    output DMA per query group.
"""

import numpy as np
import ml_dtypes

import concourse.bass as bass
import concourse.mybir as mybir
import concourse.tile as tile
from concourse import bacc
from concourse import bass_utils
from concourse.bass import ts

BF = mybir.dt.bfloat16
F32 = mybir.dt.float32

B, T, C = 2, 2048, 1024
H, DH = 16, 64
N_CORES = 8
HG = 4  # heads per core
CH = HG * DH  # 256 channels per core

LAST_RESULT = None  # BassKernelResults of the most recent run (for profiling)
_NC_CACHE = None


def _build_nc():
    nc = bacc.Bacc(
        "TRN2", target_bir_lowering=False, debug=False, num_devices=N_CORES
    )

    # host-pre-blocked layouts: every tensor is contiguous along its SBUF
    # partition's free dim, so each DMA is 128 x (one fat line).
    F8 = mybir.dt.float8e4
    xt = nc.dram_tensor("xt", [128, 4, 8, 512], BF, kind="ExternalInput")
    xq8 = nc.dram_tensor("xq8", [128, 4, 4, 2, 512], F8, kind="ExternalInput")
    wq = nc.dram_tensor("wq", [128, 4, 2, CH], F8, kind="ExternalInput")
    wk = nc.dram_tensor("wk", [128, 4, 2, CH], F8, kind="ExternalInput")
    wv = nc.dram_tensor("wv", [128, 8, CH], BF, kind="ExternalInput")
    bqt = nc.dram_tensor("bqt", [128, 2], F32, kind="ExternalInput")
    bkt = nc.dram_tensor("bkt", [128, 2], F32, kind="ExternalInput")
    wout = nc.dram_tensor("wout", [128, 2, C], BF, kind="ExternalInput")
    out = nc.dram_tensor("out", [T, C], BF, kind="ExternalOutput")

    with tile.TileContext(nc) as tc:
        with (
            tc.tile_pool(name="persist", bufs=1) as persist,
            tc.tile_pool(name="consts", bufs=1) as consts,
            tc.tile_pool(name="sbn", bufs=2) as sbn,
            tc.tile_pool(name="osb", bufs=3) as osb,
            tc.tile_pool(name="ps_st", bufs=2, space="PSUM") as ps_st,
            tc.tile_pool(name="ps_pv", bufs=1, space="PSUM") as ps_pv,
            tc.tile_pool(name="ps_misc", bufs=3, space="PSUM") as ps_misc,
        ):
            ones_bf = consts.tile([1, 128], BF)
            nc.vector.memset(ones_bf[:], 1.0)
            ones_col = consts.tile([128, 1], BF)
            nc.vector.memset(ones_col[:], 1.0)

            # fp8 x (DoubleRow-packed) for q/k; bf16 x arrives per-tt into
            # a rotating buffer for the V projections.
            x8_sb = persist.tile([128, 4, 4, 2, 512], F8, tag="x8")
            wq_sb = persist.tile([128, 4, 2, CH], F8, tag="wq")
            wk_sb = persist.tile([128, 4, 2, CH], F8, tag="wk")
            wv_sb = persist.tile([128, 8, CH], BF, tag="wv")
            wout_sb = persist.tile([128, 2, C], BF, tag="wout")
            bqt_sb = consts.tile([128, 2], F32)
            bkt_sb = consts.tile([128, 2], F32)

            # ---- input DMA: 3-way queue split, critical-first ----
            # per-queue bandwidth is only ~65-160GB/s, so the critical
            # tensors (x8-tt0 / wk / wq, all small now) lead their queues.
            nc.sync.dma_start(out=x8_sb[:, 0], in_=xq8[:, 0])
            nc.scalar.dma_start(out=wk_sb[:], in_=wk[:])
            nc.scalar.dma_start(out=bkt_sb[:], in_=bkt[:])
            nc.scalar.dma_start(out=bqt_sb[:], in_=bqt[:])
            nc.scalar.dma_start(out=wq_sb[:], in_=wq[:])
            nc.sync.dma_start(out=x8_sb[:, 1], in_=xq8[:, 1])
            nc.sync.dma_start(out=x8_sb[:, 2], in_=xq8[:, 2])
            nc.sync.dma_start(out=x8_sb[:, 3], in_=xq8[:, 3])

            # ---- PE warmup: ~6us of dummy matmuls during the DMA wait so
            # the HAM clock gate reaches 8/8 before the real projections.
            warm_src = consts.tile([128, 512], BF)
            nc.vector.memset(warm_src[:], 0.0)
            warm_ps = ps_misc.tile([128, 512], F32, tag="sm", name="warm")
            for i in range(14):
                nc.tensor.matmul(
                    warm_ps[0:1, :], warm_src[:, 0:1], warm_src[:],
                    start=(i == 0), stop=(i == 13),
                )

            # ---- persistent activations ----
            # qkT[:, 0:2, :] = qT chunks (hp), [:, 2:4, :] = kT chunks;
            # chunk hp rows 0-63 = head 2hp, rows 64-127 = head 2hp+1.
            qkT = persist.tile([128, 4, T], BF, tag="qkT")
            vext = persist.tile([128, T // 128, HG, DH], BF, tag="vext")
            attn_p = [
                [
                    persist.tile(
                        [128, 512], BF, tag=f"attnp{hp}_{qg}",
                        name=f"attnp{hp}_{qg}",
                    )
                    for qg in range(4)
                ]
                for hp in range(2)
            ]

            def qk_group(w_i, co, tt):
                """one [128,512] tile of qT (w_i=0) or kT (w_i=1), chunk co"""
                wsb = wq_sb if w_i == 0 else wk_sb
                bias_sb = bqt_sb if w_i == 0 else bkt_sb
                qp = ps_misc.tile([128, 512], F32, tag="sm", name="qp")
                for jp in range(4):
                    nc.tensor.matmul(
                        qp[:],
                        wsb[:, jp, :, ts(co, 128)],
                        x8_sb[:, tt, jp, :, :],
                        perf_mode=mybir.MatmulPerfMode.DoubleRow,
                        start=(jp == 0),
                        stop=(jp == 3),
                    )
                # bias-add + cast on the DVE (keeps the ACT queue for exps)
                nc.vector.tensor_scalar_add(
                    qkT[:, 2 * w_i + co, ts(tt, 512)],
                    qp[:],
                    bias_sb[:, co : co + 1],
                )

            def v_group(tv):
                vp = ps_misc.tile([128, CH], F32, tag="sm", name="vp")
                for ci in range(8):
                    nc.tensor.matmul(
                        vp[:],
                        xv_tiles[tv // 4][:, ci, ts(tv % 4, 128)],
                        wv_sb[:, ci, :],
                        start=(ci == 0),
                        stop=(ci == 7),
                    )
                nc.vector.tensor_copy(
                    vext[:, tv, :, :],
                    vp[:].rearrange("p (h d) -> p h d", h=HG),
                )

            p_tiles = {}
            rec_tiles = {}
            tmp_tiles = {}

            def p_alloc(qg, hp):
                # paired tile: [p, head(A/B), kp, 1024]; one pool slot per
                # round -> bufs=3 keeps 3 rounds of exps live.
                p = osb.tile([128, 2, 8, 1024], BF, tag="p", bufs=3, name="p")
                p_tiles[(qg, hp)] = p
                return p

            def st_seg(qg, hp, kps, p):
                """scores^T + exp for head pair hp, query group qg, kp range."""
                qs = ts(qg, 512)
                for kp in kps:
                    stA = ps_st.tile([128, 1024], F32, tag="st", name="stA")
                    stB = ps_st.tile([128, 1024], F32, tag="st", name="stB")
                    for j in range(2):
                        ki = 2 * kp + j
                        nc.tensor.matmul(
                            stA[:, ts(j, 512)],
                            qkT[0:64, 2 + hp, ts(ki, 128)],
                            qkT[0:64, hp, qs],
                            start=True, stop=True,
                        )
                        nc.tensor.matmul(
                            stB[:, ts(j, 512)],
                            qkT[64:128, 2 + hp, ts(ki, 128)],
                            qkT[64:128, hp, qs],
                            start=True, stop=True,
                        )
                    nc.scalar.activation(
                        p[:, 0, kp, :], stA[:],
                        mybir.ActivationFunctionType.Exp, scale=1.0 / 8.0,
                    )
                    nc.scalar.activation(
                        p[:, 1, kp, :], stB[:],
                        mybir.ActivationFunctionType.Exp, scale=1.0 / 8.0,
                    )

            def st_part(qg, hp):
                p = p_alloc(qg, hp)
                st_seg(qg, hp, range(8), p)

            def pv_part(qg, hp):
                p = p_tiles.pop((qg, hp))
                # denominator add-trees first: they depend only on the exps,
                # so emitting them ahead of the PV matmuls keeps the DVE
                # queue from head-blocking on the PV-dependent tmp copy.
                t2_eng = nc.vector
                t4s = {}
                for hh in range(2):
                    t1 = sbn.tile([128, 4, 1024], BF, tag="t1", name="t1", bufs=1)
                    nc.vector.tensor_add(
                        t1[:], p[:, hh, 0:4, :], p[:, hh, 4:8, :]
                    )
                    t2 = sbn.tile([128, 2, 1024], BF, tag="t2", name="t2", bufs=1)
                    t2_eng.tensor_add(
                        t2[:], t1[:, 0:2, :], t1[:, 2:4, :]
                    )
                    t3 = sbn.tile([128, 1024], BF, tag="t3", name="t3", bufs=1)
                    nc.vector.tensor_add(
                        t3[:], t2[:, 0, :], t2[:, 1, :]
                    )
                    t4 = sbn.tile([128, 512], BF, tag="t4", name="t4", bufs=2)
                    nc.vector.tensor_add(
                        t4[:], t3[:, 0:512], t3[:, 512:1024]
                    )
                    t4s[hh] = t4
                # paired PV: head 2hp -> psum partitions 0-63 (col group 0-1),
                # head 2hp+1 -> partitions 64-127 (col group 2-3); the two
                # column-tiled matmul streams run concurrently on the PE.
                pv = ps_pv.tile([128, 512], F32, tag="pv", name="pv")
                for ki in range(16):
                    for hh in range(2):
                        h = 2 * hp + hh
                        nc.tensor.matmul(
                            pv[64 * hh : 64 * hh + 64, :],
                            vext[:, ki, h, :],
                            p[:, hh, ki // 2, ts(ki % 2, 512)],
                            start=(ki == 0),
                            stop=(ki == 15),
                        )
                # partition-axis fold of the partial denominators (K=128
                # ones-matmul), then the reciprocal chain (DVE-only)
                recs = []
                for hh in range(2):
                    dps = ps_misc.tile([128, 512], F32, tag="sm", name="dps")
                    nc.tensor.matmul(
                        dps[0:1, :], ones_col[:, 0:1], t4s[hh][:],
                        start=True, stop=True,
                    )
                    rec32 = sbn.tile([1, 512], F32, tag="rec32", name="rc", bufs=1)
                    nc.vector.reciprocal_approx_fast(out=rec32[:], in_=dps[0:1, :])
                    rec_bf = sbn.tile([1, 512], BF, tag="rec", name="rb", bufs=4)
                    nc.vector.tensor_copy(rec_bf[:], rec32[:])
                    recs.append(rec_bf)
                rec_tiles[(qg, hp)] = recs
                tmp = sbn.tile([128, 512], BF, tag="tmp", name="tmp", bufs=3)
                # final rounds' tmp copies on ScalarE: ACT is idle once the
                # exps end, and it keeps the tail off the backlogged DVE.
                tmp_eng = nc.scalar if qg == 3 else nc.vector
                if tmp_eng is nc.scalar:
                    nc.scalar.copy(tmp[:], pv[:])
                else:
                    nc.vector.tensor_copy(tmp[:], pv[:])
                tmp_tiles[(qg, hp)] = tmp

            def normalize_round(qg, hp):
                """rep-matmul + multiply -> attn_p[hp][qg] (both heads)."""
                rp = ps_misc.tile([128, 512], F32, tag="sm", name="rp")
                tmp = tmp_tiles.pop((qg, hp))
                recs = rec_tiles.pop((qg, hp))
                for hh in range(2):
                    rows = slice(64 * hh, 64 * hh + 64)
                    nc.tensor.matmul(
                        rp[rows, :], ones_bf[0:1, 0:64], recs[hh][:],
                        start=True, stop=True,
                    )
                nc.vector.tensor_mul(
                    attn_p[hp][qg][:], tmp[:], rp[:],
                )

            def outproj_chunk(qg):
                """partial out-projection rows for query group qg."""
                for tt4 in range(4):
                    tt = 4 * qg + tt4
                    o_sb = osb.tile([128, C], BF, tag="o", name="osb", bufs=2)
                    for cn in range(2):
                        op = ps_misc.tile(
                            [128, 512], F32, tag="sm", name="op"
                        )
                        for hp in range(2):
                            nc.tensor.matmul(
                                op[:],
                                attn_p[hp][qg][:, ts(tt4, 128)],
                                wout_sb[:, hp, ts(cn, 512)],
                                start=(hp == 0),
                                stop=(hp == 1),
                            )
                        if qg >= 2:
                            nc.scalar.copy(o_sb[:, ts(cn, 512)], op[:])
                        else:
                            nc.vector.tensor_copy(o_sb[:, ts(cn, 512)], op[:])
                    # alternate output queues; bf16 halves the bytes so the
                    # last chunk's transfer is ~2us instead of ~4.4us.
                    oq = nc.sync if tt % 2 == 0 else nc.gpsimd
                    oq.dma_start(out=out[ts(tt, 128), :], in_=o_sb[:])

            # ---- flash-style startup: feed ACT as early as possible ----
            # Scores-critical work is emitted (= prioritized) strictly ahead
            # of the V projections, which are pure PE filler in the ACT-paced
            # slack of rounds 1-2; pv(0,0) directly follows V.
            # round 0 (qg0, hp0): k chunk0 + q chunk0(tt0); scores chase the
            # k tt-groups as they land.
            qk_group(1, 0, 0)
            qk_group(0, 0, 0)
            # bulk loads (4MB bf16 x for V + wv + wout) are gated behind
            # round-0 data: all queues share ~330GB/s, so these must not
            # compete with the critical fp8/weight loads.
            defer_sb = consts.tile([1, 64], BF)
            nc.gpsimd.tensor_copy(defer_sb[:], qkT[0:1, 2, 0:64])
            nc.gpsimd.dma_start(out=wv_sb[:], in_=wv[:])
            xv_tiles = []
            for vtt in range(4):
                xv = osb.tile([128, 8, 512], BF, tag="xv", name="xv", bufs=2)
                nc.gpsimd.dma_start(out=xv[:], in_=xt[:, vtt])
                xv_tiles.append(xv)
            nc.gpsimd.dma_start(out=wout_sb[:], in_=wout[:])
            p00 = p_alloc(0, 0)
            st_seg(0, 0, [0, 1], p00)
            qk_group(1, 0, 1)
            st_seg(0, 0, [2, 3], p00)
            qk_group(1, 0, 2)
            st_seg(0, 0, [4, 5], p00)
            qk_group(1, 0, 3)
            st_seg(0, 0, [6, 7], p00)

            # round 1 (qg0, hp1): k chunk1 + q chunk1(tt0)
            qk_group(1, 1, 0)
            qk_group(0, 1, 0)
            p01 = p_alloc(0, 1)
            st_seg(0, 1, [0, 1], p01)
            qk_group(1, 1, 1)
            st_seg(0, 1, [2, 3], p01)
            qk_group(1, 1, 2)
            st_seg(0, 1, [4, 5], p01)
            qk_group(1, 1, 3)
            st_seg(0, 1, [6, 7], p01)

            # round 2 (qg1, hp0) scores, then V in its ACT-slack
            qk_group(0, 0, 1)
            qk_group(0, 1, 1)
            st_part(1, 0)
            for tv in range(16):
                v_group(tv)

            # ---- pipelined main stream ----
            pv_part(0, 0)
            st_part(1, 1)
            pv_part(0, 1)
            qk_group(0, 0, 2)
            qk_group(0, 1, 2)
            st_part(2, 0)
            normalize_round(0, 0)
            pv_part(1, 0)
            st_part(2, 1)
            normalize_round(0, 1)
            outproj_chunk(0)
            pv_part(1, 1)
            qk_group(0, 0, 3)
            qk_group(0, 1, 3)
            st_part(3, 0)
            normalize_round(1, 0)
            pv_part(2, 0)
            st_part(3, 1)
            normalize_round(1, 1)
            outproj_chunk(1)
            pv_part(2, 1)
            normalize_round(2, 0)
            pv_part(3, 0)
            normalize_round(2, 1)
            outproj_chunk(2)
            pv_part(3, 1)
            normalize_round(3, 0)
            normalize_round(3, 1)
            outproj_chunk(3)

    nc.compile()
    return nc


def _get_nc():
    global _NC_CACHE
    if _NC_CACHE is None:
        _NC_CACHE = _build_nc()
    return _NC_CACHE


def kernel(x, w_qkv, b_qkv, w_out, b_out):
    global LAST_RESULT
    x = np.asarray(x, dtype=np.float32)
    w_qkv = np.asarray(w_qkv, dtype=np.float32)
    b_qkv = np.asarray(b_qkv, dtype=np.float32)
    w_out = np.asarray(w_out, dtype=np.float32)
    b_out = np.asarray(b_out, dtype=np.float32)

    bf = ml_dtypes.bfloat16

    f8 = ml_dtypes.float8_e4m3fn

    def blk_w(w):  # [1024, n] -> [128, 8, n] (p, ci, n) contiguous
        n = w.shape[1]
        return np.ascontiguousarray(
            w.reshape(8, 128, n).transpose(1, 0, 2)
        ).astype(bf)

    def blk_w8(w):  # [1024, n] -> [128, 4, 2, n] DoubleRow-packed fp8
        n = w.shape[1]
        return np.ascontiguousarray(
            w.reshape(4, 2, 128, n).transpose(2, 0, 1, 3).astype(f8)
        )

    in_maps = []
    for c in range(N_CORES):
        b, g = divmod(c, 4)
        cols = slice(CH * g, CH * (g + 1))
        bq = b_qkv[0 * C + CH * g : 0 * C + CH * (g + 1)]
        bk = b_qkv[1 * C + CH * g : 1 * C + CH * (g + 1)]
        # x^T token-blocked: [p, tt, ci, 512]
        xtb = np.ascontiguousarray(
            x[b].T.astype(bf).reshape(8, 128, 4, 512).transpose(1, 2, 0, 3)
        )
        # fp8 x^T DoubleRow-packed: [p, tt, jp, ko, 512]
        x8b = np.ascontiguousarray(
            x[b].T.reshape(4, 2, 128, 4, 512).transpose(2, 3, 0, 1, 4).astype(f8)
        )
        # wout row-blocked: [p, hp, 1024]
        wob = np.ascontiguousarray(
            w_out[CH * g : CH * (g + 1), :].reshape(2, 128, C).transpose(1, 0, 2)
        ).astype(bf)
        in_maps.append(
            {
                "xt": xtb,
                "xq8": x8b,
                "wq": blk_w8(w_qkv[:, 0 * C :][:, cols]),
                "wk": blk_w8(w_qkv[:, 1 * C :][:, cols]),
                "wv": blk_w(w_qkv[:, 2 * C :][:, cols]),
                "bqt": np.ascontiguousarray(bq.reshape(2, 128).T),
                "bkt": np.ascontiguousarray(bk.reshape(2, 128).T),
                "wout": wob,
            }
        )

    nc = _get_nc()
    LAST_RESULT = bass_utils.run_bass_kernel_spmd(
        nc, in_maps, core_ids=list(range(N_CORES))
    )

    full = np.zeros((B, T, C), dtype=np.float32)
    # bias folded once on the host: b_out plus the V-bias pushed through
    # w_out (normalized attention rows sum to 1, so bv contributes exactly
    # bv @ w_out to every token)
    full += b_out + b_qkv[2 * C : 3 * C] @ w_out
    for c in range(N_CORES):
        b = c // 4
        full[b] += LAST_RESULT.results[c]["out"].astype(np.float32)
    return full


# revision 36
# speedup vs baseline: 1.3367x; 1.0091x over previous
"""Multi-head self-attention on 8 Trainium2 NeuronCores (Bass/Tile).

Problem: x[2,2048,1024] -> MHA(16 heads, d_head 64) -> out[2,2048,1024].

Sharding (batch x head-group, Megatron-ish, collective-free):
  core c (0..7): batch b = c//4, head group g = c%4 (heads 4g..4g+3).
  Each core computes q/k/v projections for its 4 heads over its batch,
  attention for those heads, and a PARTIAL output projection
  attn_local[256ch] @ w_out[256ch rows] over the full sequence. The host
  sums the 4 partials per batch (the Megatron row-parallel all-reduce is
  folded into the unshard step; b_out and the V-bias term bv @ w_out are
  added once on the host -- exact, since softmax rows sum to 1).

On-core layout (TensorE compute in bf16, fp32 PSUM accumulation):
  - ACT (exp for softmax) is the bottleneck engine: 16.8M exps/core ~=
    147us of ACT instruction time. The schedule saturates ACT from the
    earliest possible point after the ~7us engine-boot preamble:
    * all inputs arrive in host-pre-blocked layouts that are contiguous
      per SBUF partition (cheap HWDGE triggers, 4-8KB DMA lines), split
      across the sync and scalar queues with x^T token-sliced;
    * k/q chunk-0 projections are emitted first so scores round 0 feeds
      ACT immediately; V projections and the remaining q/k chunks are
      pure PE filler behind ACT pacing.
  - exp p-tiles are paired per round ([128, 2heads, 8kp, 1024]) with 3
    rotating buffers = 3 rounds in flight, so round r+2's exps never
    wait on round r's PV consumption.
  - qT/kT in [channel, t] layout: scores^T = kT.T @ qT with the two
    heads of a chunk in partitions 0-63/64-127 -> concurrent K=64
    matmuls in disjoint PE row groups.
  - softmax: scores^T [128ki, qi] -> ACT exp (PSUM->SBUF bf16,
    scale=1/8 folded, no max subtraction: |s|/8 <= ~2).
  - PV: attn^T = V.T @ P~ as column-tiled concurrent M=64 head pairs;
    denominators via DVE bf16 add-tree + K=128 ones-matmul fold,
    reciprocal_approx_fast straight off PSUM; normalize one round
    behind PV (rep-matmul broadcast + DVE mul); out-projection +
    output DMA per query group.
"""

import numpy as np
import ml_dtypes

import concourse.bass as bass
import concourse.mybir as mybir
import concourse.tile as tile
from concourse import bacc
from concourse import bass_utils
from concourse.bass import ts

BF = mybir.dt.bfloat16
F32 = mybir.dt.float32

B, T, C = 2, 2048, 1024
H, DH = 16, 64
N_CORES = 8
HG = 4  # heads per core
CH = HG * DH  # 256 channels per core

LAST_RESULT = None  # BassKernelResults of the most recent run (for profiling)
_NC_CACHE = None


def _build_nc():
    nc = bacc.Bacc(
        "TRN2", target_bir_lowering=False, debug=False, num_devices=N_CORES
    )

    # host-pre-blocked layouts: every tensor is contiguous along its SBUF
    # partition's free dim, so each DMA is 128 x (one fat line).
    F8 = mybir.dt.float8e4
    xt = nc.dram_tensor("xt", [128, 4, 8, 512], BF, kind="ExternalInput")
    xq8 = nc.dram_tensor("xq8", [128, 4, 4, 2, 512], F8, kind="ExternalInput")
    wq = nc.dram_tensor("wq", [128, 4, 2, CH], F8, kind="ExternalInput")
    wk = nc.dram_tensor("wk", [128, 4, 2, CH], F8, kind="ExternalInput")
    wv = nc.dram_tensor("wv", [128, 8, CH], BF, kind="ExternalInput")
    bqt = nc.dram_tensor("bqt", [128, 2], F32, kind="ExternalInput")
    bkt = nc.dram_tensor("bkt", [128, 2], F32, kind="ExternalInput")
    wout = nc.dram_tensor("wout", [128, 2, C], BF, kind="ExternalInput")
    out = nc.dram_tensor("out", [T, C], BF, kind="ExternalOutput")

    with tile.TileContext(nc) as tc:
        with (
            tc.tile_pool(name="persist", bufs=1) as persist,
            tc.tile_pool(name="consts", bufs=1) as consts,
            tc.tile_pool(name="sbn", bufs=2) as sbn,
            tc.tile_pool(name="osb", bufs=3) as osb,
            tc.tile_pool(name="ps_st", bufs=2, space="PSUM") as ps_st,
            tc.tile_pool(name="ps_pv", bufs=1, space="PSUM") as ps_pv,
            tc.tile_pool(name="ps_misc", bufs=3, space="PSUM") as ps_misc,
        ):
            ones_bf = consts.tile([1, 128], BF)
            nc.vector.memset(ones_bf[:], 1.0)
            ones_col = consts.tile([128, 1], BF)
            nc.vector.memset(ones_col[:], 1.0)

            # fp8 x (DoubleRow-packed) for q/k; bf16 x arrives per-tt into
            # a rotating buffer for the V projections.
            x8_sb = persist.tile([128, 4, 4, 2, 512], F8, tag="x8")
            wq_sb = persist.tile([128, 4, 2, CH], F8, tag="wq")
            wk_sb = persist.tile([128, 4, 2, CH], F8, tag="wk")
            wv_sb = persist.tile([128, 8, CH], BF, tag="wv")
            wout_sb = persist.tile([128, 2, C], BF, tag="wout")
            bqt_sb = consts.tile([128, 2], F32)
            bkt_sb = consts.tile([128, 2], F32)

            # ---- input DMA: 3-way queue split, critical-first ----
            # per-queue bandwidth is only ~65-160GB/s, so the critical
            # tensors (x8-tt0 / wk / wq, all small now) lead their queues.
            # everything critical rides the fast sync queue: round-robin
            # striping finishes the small tensors first, then the x8 slices.
            nc.sync.dma_start(out=bkt_sb[:], in_=bkt[:])
            nc.sync.dma_start(out=bqt_sb[:], in_=bqt[:])
            nc.sync.dma_start(out=wk_sb[:], in_=wk[:])
            nc.sync.dma_start(out=wq_sb[:], in_=wq[:])
            nc.sync.dma_start(out=x8_sb[:, 0], in_=xq8[:, 0])
            nc.sync.dma_start(out=x8_sb[:, 1], in_=xq8[:, 1])
            nc.sync.dma_start(out=x8_sb[:, 2], in_=xq8[:, 2])
            nc.sync.dma_start(out=x8_sb[:, 3], in_=xq8[:, 3])

            # ---- PE warmup: ~6us of dummy matmuls during the DMA wait so
            # the HAM clock gate reaches 8/8 before the real projections.
            warm_src = consts.tile([128, 512], BF)
            nc.vector.memset(warm_src[:], 0.0)
            warm_ps = ps_misc.tile([128, 512], F32, tag="sm", name="warm")
            for i in range(14):
                nc.tensor.matmul(
                    warm_ps[0:1, :], warm_src[:, 0:1], warm_src[:],
                    start=(i == 0), stop=(i == 13),
                )

            # ---- persistent activations ----
            # qkT[:, 0:2, :] = qT chunks (hp), [:, 2:4, :] = kT chunks;
            # chunk hp rows 0-63 = head 2hp, rows 64-127 = head 2hp+1.
            qkT = persist.tile([128, 4, T], BF, tag="qkT")
            vext = persist.tile([128, T // 128, HG, DH], BF, tag="vext")
            attn_p = [
                [
                    persist.tile(
                        [128, 512], BF, tag=f"attnp{hp}_{qg}",
                        name=f"attnp{hp}_{qg}",
                    )
                    for qg in range(4)
                ]
                for hp in range(2)
            ]

            def qk_group(w_i, co, tt):
                """one [128,512] tile of qT (w_i=0) or kT (w_i=1), chunk co"""
                wsb = wq_sb if w_i == 0 else wk_sb
                bias_sb = bqt_sb if w_i == 0 else bkt_sb
                qp = ps_misc.tile([128, 512], F32, tag="sm", name="qp")
                for jp in range(4):
                    nc.tensor.matmul(
                        qp[:],
                        wsb[:, jp, :, ts(co, 128)],
                        x8_sb[:, tt, jp, :, :],
                        perf_mode=mybir.MatmulPerfMode.DoubleRow,
                        start=(jp == 0),
                        stop=(jp == 3),
                    )
                # bias-add + cast on the DVE (keeps the ACT queue for exps)
                nc.vector.tensor_scalar_add(
                    qkT[:, 2 * w_i + co, ts(tt, 512)],
                    qp[:],
                    bias_sb[:, co : co + 1],
                )

            def v_group(tv):
                vp = ps_misc.tile([128, CH], F32, tag="sm", name="vp")
                for ci in range(8):
                    nc.tensor.matmul(
                        vp[:],
                        xv_tiles[tv // 4][:, ci, ts(tv % 4, 128)],
                        wv_sb[:, ci, :],
                        start=(ci == 0),
                        stop=(ci == 7),
                    )
                nc.vector.tensor_copy(
                    vext[:, tv, :, :],
                    vp[:].rearrange("p (h d) -> p h d", h=HG),
                )

            p_tiles = {}
            rec_tiles = {}
            tmp_tiles = {}

            def p_alloc(qg, hp):
                # paired tile: [p, head(A/B), kp, 1024]; one pool slot per
                # round -> bufs=3 keeps 3 rounds of exps live.
                p = osb.tile([128, 2, 8, 1024], BF, tag="p", bufs=3, name="p")
                p_tiles[(qg, hp)] = p
                return p

            def st_seg(qg, hp, kps, p):
                """scores^T + exp for head pair hp, query group qg, kp range."""
                qs = ts(qg, 512)
                for kp in kps:
                    stA = ps_st.tile([128, 1024], F32, tag="st", name="stA")
                    stB = ps_st.tile([128, 1024], F32, tag="st", name="stB")
                    for j in range(2):
                        ki = 2 * kp + j
                        nc.tensor.matmul(
                            stA[:, ts(j, 512)],
                            qkT[0:64, 2 + hp, ts(ki, 128)],
                            qkT[0:64, hp, qs],
                            start=True, stop=True,
                        )
                        nc.tensor.matmul(
                            stB[:, ts(j, 512)],
                            qkT[64:128, 2 + hp, ts(ki, 128)],
                            qkT[64:128, hp, qs],
                            start=True, stop=True,
                        )
                    nc.scalar.activation(
                        p[:, 0, kp, :], stA[:],
                        mybir.ActivationFunctionType.Exp, scale=1.0 / 8.0,
                    )
                    nc.scalar.activation(
                        p[:, 1, kp, :], stB[:],
                        mybir.ActivationFunctionType.Exp, scale=1.0 / 8.0,
                    )

            def st_part(qg, hp):
                p = p_alloc(qg, hp)
                st_seg(qg, hp, range(8), p)

            def pv_part(qg, hp):
                p = p_tiles.pop((qg, hp))
                # denominator add-trees first: they depend only on the exps,
                # so emitting them ahead of the PV matmuls keeps the DVE
                # queue from head-blocking on the PV-dependent tmp copy.
                t2_eng = nc.vector
                t4s = {}
                for hh in range(2):
                    t1 = sbn.tile([128, 4, 1024], BF, tag="t1", name="t1", bufs=1)
                    nc.vector.tensor_add(
                        t1[:], p[:, hh, 0:4, :], p[:, hh, 4:8, :]
                    )
                    t2 = sbn.tile([128, 2, 1024], BF, tag="t2", name="t2", bufs=1)
                    t2_eng.tensor_add(
                        t2[:], t1[:, 0:2, :], t1[:, 2:4, :]
                    )
                    t3 = sbn.tile([128, 1024], BF, tag="t3", name="t3", bufs=1)
                    nc.vector.tensor_add(
                        t3[:], t2[:, 0, :], t2[:, 1, :]
                    )
                    t4 = sbn.tile([128, 512], BF, tag="t4", name="t4", bufs=2)
                    nc.vector.tensor_add(
                        t4[:], t3[:, 0:512], t3[:, 512:1024]
                    )
                    t4s[hh] = t4
                # paired PV: head 2hp -> psum partitions 0-63 (col group 0-1),
                # head 2hp+1 -> partitions 64-127 (col group 2-3); the two
                # column-tiled matmul streams run concurrently on the PE.
                pv = ps_pv.tile([128, 512], F32, tag="pv", name="pv")
                for ki in range(16):
                    for hh in range(2):
                        h = 2 * hp + hh
                        nc.tensor.matmul(
                            pv[64 * hh : 64 * hh + 64, :],
                            vext[:, ki, h, :],
                            p[:, hh, ki // 2, ts(ki % 2, 512)],
                            start=(ki == 0),
                            stop=(ki == 15),
                        )
                # partition-axis fold of the partial denominators (K=128
                # ones-matmul), then the reciprocal chain (DVE-only)
                recs = []
                for hh in range(2):
                    dps = ps_misc.tile([128, 512], F32, tag="sm", name="dps")
                    nc.tensor.matmul(
                        dps[0:1, :], ones_col[:, 0:1], t4s[hh][:],
                        start=True, stop=True,
                    )
                    rec32 = sbn.tile([1, 512], F32, tag="rec32", name="rc", bufs=1)
                    nc.vector.reciprocal_approx_fast(out=rec32[:], in_=dps[0:1, :])
                    rec_bf = sbn.tile([1, 512], BF, tag="rec", name="rb", bufs=4)
                    nc.vector.tensor_copy(rec_bf[:], rec32[:])
                    recs.append(rec_bf)
                rec_tiles[(qg, hp)] = recs
                tmp = sbn.tile([128, 512], BF, tag="tmp", name="tmp", bufs=3)
                # final rounds' tmp copies on ScalarE: ACT is idle once the
                # exps end, and it keeps the tail off the backlogged DVE.
                tmp_eng = nc.scalar if qg == 3 else nc.vector
                if tmp_eng is nc.scalar:
                    nc.scalar.copy(tmp[:], pv[:])
                else:
                    nc.vector.tensor_copy(tmp[:], pv[:])
                tmp_tiles[(qg, hp)] = tmp

            def normalize_round(qg, hp):
                """rep-matmul + multiply -> attn_p[hp][qg] (both heads)."""
                rp = ps_misc.tile([128, 512], F32, tag="sm", name="rp")
                tmp = tmp_tiles.pop((qg, hp))
                recs = rec_tiles.pop((qg, hp))
                for hh in range(2):
                    rows = slice(64 * hh, 64 * hh + 64)
                    nc.tensor.matmul(
                        rp[rows, :], ones_bf[0:1, 0:64], recs[hh][:],
                        start=True, stop=True,
                    )
                nc.vector.tensor_mul(
                    attn_p[hp][qg][:], tmp[:], rp[:],
                )

            def outproj_chunk(qg):
                """partial out-projection rows for query group qg."""
                for tt4 in range(4):
                    tt = 4 * qg + tt4
                    o_sb = osb.tile([128, C], BF, tag="o", name="osb", bufs=2)
                    for cn in range(2):
                        op = ps_misc.tile(
                            [128, 512], F32, tag="sm", name="op"
                        )
                        for hp in range(2):
                            nc.tensor.matmul(
                                op[:],
                                attn_p[hp][qg][:, ts(tt4, 128)],
                                wout_sb[:, hp, ts(cn, 512)],
                                start=(hp == 0),
                                stop=(hp == 1),
                            )
                        if qg >= 2:
                            nc.scalar.copy(o_sb[:, ts(cn, 512)], op[:])
                        else:
                            nc.vector.tensor_copy(o_sb[:, ts(cn, 512)], op[:])
                    # alternate output queues; bf16 halves the bytes so the
                    # last chunk's transfer is ~2us instead of ~4.4us.
                    oq = nc.scalar if tt % 2 == 0 else nc.gpsimd
                    oq.dma_start(out=out[ts(tt, 128), :], in_=o_sb[:])

            # ---- flash-style startup: feed ACT as early as possible ----
            # Scores-critical work is emitted (= prioritized) strictly ahead
            # of the V projections, which are pure PE filler in the ACT-paced
            # slack of rounds 1-2; pv(0,0) directly follows V.
            # round 0 (qg0, hp0): k chunk0 + q chunk0(tt0); scores chase the
            # k tt-groups as they land.
            qk_group(1, 0, 0)
            qk_group(0, 0, 0)
            # bulk loads (4MB bf16 x for V + wv + wout) are gated behind
            # round-0 data: all queues share ~330GB/s, so these must not
            # compete with the critical fp8/weight loads.
            defer_sb = consts.tile([1, 64], BF)
            nc.gpsimd.tensor_copy(defer_sb[:], qkT[0:1, 2, 0:64])
            nc.gpsimd.dma_start(out=wv_sb[:], in_=wv[:])
            xv_tiles = []
            for vtt in range(4):
                xv = osb.tile([128, 8, 512], BF, tag="xv", name="xv", bufs=2)
                nc.gpsimd.dma_start(out=xv[:], in_=xt[:, vtt])
                xv_tiles.append(xv)
            nc.gpsimd.dma_start(out=wout_sb[:], in_=wout[:])
            p00 = p_alloc(0, 0)
            st_seg(0, 0, [0, 1], p00)
            qk_group(1, 0, 1)
            st_seg(0, 0, [2, 3], p00)
            qk_group(1, 0, 2)
            st_seg(0, 0, [4, 5], p00)
            qk_group(1, 0, 3)
            st_seg(0, 0, [6, 7], p00)

            # round 1 (qg0, hp1): k chunk1 + q chunk1(tt0)
            qk_group(1, 1, 0)
            qk_group(0, 1, 0)
            p01 = p_alloc(0, 1)
            st_seg(0, 1, [0, 1], p01)
            qk_group(1, 1, 1)
            st_seg(0, 1, [2, 3], p01)
            qk_group(1, 1, 2)
            st_seg(0, 1, [4, 5], p01)
            qk_group(1, 1, 3)
            st_seg(0, 1, [6, 7], p01)

            # round 2 (qg1, hp0) scores, then V in its ACT-slack
            qk_group(0, 0, 1)
            qk_group(0, 1, 1)
            st_part(1, 0)
            for tv in range(16):
                v_group(tv)

            # ---- pipelined main stream ----
            pv_part(0, 0)
            st_part(1, 1)
            pv_part(0, 1)
            qk_group(0, 0, 2)
            qk_group(0, 1, 2)
            st_part(2, 0)
            normalize_round(0, 0)
            pv_part(1, 0)
            st_part(2, 1)
            normalize_round(0, 1)
            outproj_chunk(0)
            pv_part(1, 1)
            qk_group(0, 0, 3)
            qk_group(0, 1, 3)
            st_part(3, 0)
            normalize_round(1, 0)
            pv_part(2, 0)
            st_part(3, 1)
            normalize_round(1, 1)
            outproj_chunk(1)
            pv_part(2, 1)
            normalize_round(2, 0)
            pv_part(3, 0)
            normalize_round(2, 1)
            outproj_chunk(2)
            pv_part(3, 1)
            normalize_round(3, 0)
            normalize_round(3, 1)
            outproj_chunk(3)

    nc.compile()
    return nc


def _get_nc():
    global _NC_CACHE
    if _NC_CACHE is None:
        _NC_CACHE = _build_nc()
    return _NC_CACHE


def kernel(x, w_qkv, b_qkv, w_out, b_out):
    global LAST_RESULT
    x = np.asarray(x, dtype=np.float32)
    w_qkv = np.asarray(w_qkv, dtype=np.float32)
    b_qkv = np.asarray(b_qkv, dtype=np.float32)
    w_out = np.asarray(w_out, dtype=np.float32)
    b_out = np.asarray(b_out, dtype=np.float32)

    bf = ml_dtypes.bfloat16

    f8 = ml_dtypes.float8_e4m3fn

    def blk_w(w):  # [1024, n] -> [128, 8, n] (p, ci, n) contiguous
        n = w.shape[1]
        return np.ascontiguousarray(
            w.reshape(8, 128, n).transpose(1, 0, 2)
        ).astype(bf)

    def blk_w8(w):  # [1024, n] -> [128, 4, 2, n] DoubleRow-packed fp8
        n = w.shape[1]
        return np.ascontiguousarray(
            w.reshape(4, 2, 128, n).transpose(2, 0, 1, 3).astype(f8)
        )

    in_maps = []
    for c in range(N_CORES):
        b, g = divmod(c, 4)
        cols = slice(CH * g, CH * (g + 1))
        bq = b_qkv[0 * C + CH * g : 0 * C + CH * (g + 1)]
        bk = b_qkv[1 * C + CH * g : 1 * C + CH * (g + 1)]
        # x^T token-blocked: [p, tt, ci, 512]
        xtb = np.ascontiguousarray(
            x[b].T.astype(bf).reshape(8, 128, 4, 512).transpose(1, 2, 0, 3)
        )
        # fp8 x^T DoubleRow-packed: [p, tt, jp, ko, 512]
        x8b = np.ascontiguousarray(
            x[b].T.reshape(4, 2, 128, 4, 512).transpose(2, 3, 0, 1, 4).astype(f8)
        )
        # wout row-blocked: [p, hp, 1024]
        wob = np.ascontiguousarray(
            w_out[CH * g : CH * (g + 1), :].reshape(2, 128, C).transpose(1, 0, 2)
        ).astype(bf)
        in_maps.append(
            {
                "xt": xtb,
                "xq8": x8b,
                "wq": blk_w8(w_qkv[:, 0 * C :][:, cols]),
                "wk": blk_w8(w_qkv[:, 1 * C :][:, cols]),
                "wv": blk_w(w_qkv[:, 2 * C :][:, cols]),
                "bqt": np.ascontiguousarray(bq.reshape(2, 128).T),
                "bkt": np.ascontiguousarray(bk.reshape(2, 128).T),
                "wout": wob,
            }
        )

    nc = _get_nc()
    LAST_RESULT = bass_utils.run_bass_kernel_spmd(
        nc, in_maps, core_ids=list(range(N_CORES))
    )

    full = np.zeros((B, T, C), dtype=np.float32)
    # bias folded once on the host: b_out plus the V-bias pushed through
    # w_out (normalized attention rows sum to 1, so bv contributes exactly
    # bv @ w_out to every token)
    full += b_out + b_qkv[2 * C : 3 * C] @ w_out
    for c in range(N_CORES):
        b = c // 4
        full[b] += LAST_RESULT.results[c]["out"].astype(np.float32)
    return full
